# revision 3
# baseline (speedup 1.0000x reference)
"""Trainium2 Bass kernel for nn_DiacriticRestorer (2-layer biLSTM encoder +
2-layer LSTM decoder + linear head), data-parallel over batch on 8 NeuronCores.

v2 design (vs baseline): the scan loop is dependency-chain bound, so the two
interleaved scans of each pair are FUSED into shared wide elementwise ops:

 - psum gates tile [P, 1024] f32 spans 2 banks: scan X gates in bank 0
   (cols 0:128), scan Y in bank 1 (cols 512:640) so each scan's start=True
   identity matmul owns its own 2KB zero region. Within a scan, gate blocks
   are ordered (f, i, g, o), each 32 cols = 4 hidden-chunks x 8 batch.
 - xg (with biases) enters psum via ONE identity-stationary matmul per scan;
   Whh chunk matmuls accumulate on top, f block first so sigmoid(f) can
   start while i/g/o matmuls still run.
 - gate math per pair-step (g rows host-pre-doubled so tanh(g)=2*sig(2g)-1):
   sig_f -> c *= sig_f; sig_ig -> tg = 2*sig_g-1, t1 = sig_i*tg, c += t1;
   sig_o (off critical path); th = tanh(c) (real Tanh table, co-resident
   with Sigmoid -> no table reloads); h = sig_o*th.
 - h goes into a rotating staging tile [P, 2*U*HW] (U=8 steps, X block then
   Y block, backward scans write slot U-1-u); xg loads and y stores are one
   DMA per U steps with 2KB contiguous rows (block-major DRAM layouts).
 - projections write all 16 gate-chunks of a token tile into one wide SBUF
   staging tile and store with a single 2KB-element DMA; y is read back one
   whole block-tile per DMA.
 - decoder pair (d0, d1 with a CH-step lag) shares the same body; head/tail
   segments feed the missing scan a zeroed xg tile and skip its y store.
"""

import numpy as np

import concourse.bacc as bacc
import concourse.bass as bass
import concourse.mybir as mybir
import concourse.tile as tile
from concourse.bass import ds
from concourse.bass_utils import run_bass_kernel_spmd
from contextlib import ExitStack

F16 = mybir.dt.float16
F32 = mybir.dt.float32
AF = mybir.ActivationFunctionType
ALU = mybir.AluOpType

V, E, H, B = 128, 256, 512, 64
NCORES = 8
BL = B // NCORES  # 8
G = 4 * H  # 2048
NK = H // 128  # 4  h chunks
NM = G // 128  # 16 gate chunks
P = 128
U = 8           # steps per staging block / per DMA batch
XW = NM * BL    # 128: xg cols per step per scan
HW = NK * BL    # 32: h cols per scan
SOFF = 512      # psum col offset of scan Y (bank 1)

# gate-chunk order in xg cols / psum blocks: [f(4), i(4), g(4), o(4)].
# PyTorch gate rows are i(m0-3), f(m4-7), g(m8-11), o(m12-15).
PERM = [4, 5, 6, 7, 0, 1, 2, 3, 8, 9, 10, 11, 12, 13, 14, 15]

WHH_DT = F16   # stationary dtype for recurrent weights
WSCALE = 1.0   # gate-path scale: weights/tables/biases packed xWSCALE,
               # sigmoid ACT ops read psum with scale=1/WSCALE (for fp8 range)


def build_model(nc, T):
    TOK = BL * T
    NTT = min(512, TOK)  # tokens per GEMM tile
    NTILES = TOK // NTT
    TT = NTT // BL  # timesteps per GEMM tile
    NB = TT // U    # staging blocks per GEMM tile (8)
    TB = T // U     # total staging blocks
    CH = TT         # decoder chunk = one proj token-tile worth of steps
    CHB = CH // U
    NCH = T // CH

    ein = lambda name, shape, dt=F16: nc.dram_tensor(
        name, shape, dt, kind="ExternalInput"
    )

    onehotT = ein("onehotT", [P, TOK])
    tab_f = ein("tab_f", [P, G])
    tab_b = ein("tab_b", [P, G])
    tab_d = ein("tab_d", [P, G])
    whh = {
        k: ein(f"whhT_{k}", [P, NK * G], WHH_DT)
        for k in ("l0f", "l0b", "l1f", "l1b", "d0", "d1")
    }
    wih_l1f = ein("wihT_l1f", [P, 8 * G])
    wih_l1b = ein("wihT_l1b", [P, 8 * G])
    wih_d0e = ein("wihT_d0e", [P, 8 * G])
    wih_d1 = ein("wihT_d1", [P, 4 * G])
    owT = ein("owT", [P, NK * V])
    ident = ein("ident", [P, P])
    bias_l1f = ein("bias_l1f", [P, NM], F32)
    bias_l1b = ein("bias_l1b", [P, NM], F32)
    bias_d1 = ein("bias_d1", [P, NM], F32)
    bias_out = ein("bias_out", [P, 1], F32)

    logitsT = nc.dram_tensor("logitsT", [P, TOK], F32, kind="ExternalOutput")

    # internal DRAM scratch, block-major: row = U consecutive steps (2KB rows)
    xg = {
        k: nc.dram_tensor(f"xg_{k}", [P, TB, U * XW], F16)
        for k in ("af", "ab", "bf", "bb")
    }
    ybuf = {
        k: nc.dram_tensor(f"y_{k}", [P, TB, U * HW], F16)
        for k in ("l0f", "l0b", "l1f", "l1b", "d0", "d1")
    }

    with tile.TileContext(nc) as tc, ExitStack() as ctx:
        const = ctx.enter_context(tc.tile_pool(name="const", bufs=1))
        wpool = ctx.enter_context(tc.tile_pool(name="wpool", bufs=1))
        whpool = ctx.enter_context(tc.tile_pool(name="whpool", bufs=1))
        ypool = ctx.enter_context(tc.tile_pool(name="ybl", bufs=4))
        pspool = ctx.enter_context(tc.tile_pool(name="ps", bufs=3, space="PSUM"))
        ps2pool = ctx.enter_context(tc.tile_pool(name="ps2", bufs=2, space="PSUM"))
        wxpool = ctx.enter_context(tc.tile_pool(name="wx", bufs=2))
        xsbpool = ctx.enter_context(tc.tile_pool(name="xsb", bufs=4))
        spool = ctx.enter_context(tc.tile_pool(name="scan", bufs=2))
        xgpool = ctx.enter_context(tc.tile_pool(name="xgp", bufs=4))
        state = ctx.enter_context(tc.tile_pool(name="state", bufs=1))

        oh_sb = const.tile([P, TOK], F16)
        nc.sync.dma_start(oh_sb[:], onehotT[:])
        bias_sb = {}
        for nm, t in (("l1f", bias_l1f), ("l1b", bias_l1b), ("d1", bias_d1)):
            bias_sb[nm] = const.tile([P, NM], F32, name=f"bias_{nm}", tag=f"bias_{nm}")
            nc.sync.dma_start(bias_sb[nm][:], t[:])
        bout_sb = const.tile([P, 1], F32)
        nc.sync.dma_start(bout_sb[:], bias_out[:])
        ident_sb = const.tile([P, P], F16)
        nc.sync.dma_start(ident_sb[:], ident[:])
        zs = const.tile([P, XW], F16, name="zs")
        nc.vector.memset(zs[:], 0.0)

        # persistent scan state: two staging buffers (h history) + cell state.
        # staging layout: X block cols [0, U*HW), Y block cols [U*HW, 2*U*HW);
        # within a block, slot w at w*HW.
        YO = U * HW  # 256
        st = [state.tile([P, 2 * U * HW], F16, name=f"st{i}", tag=f"st{i}")
              for i in range(2)]
        cT = state.tile([P, 2 * HW], F32, name="cT")
        sav_hf = state.tile([P, HW], F16)
        sav_cf = state.tile([P, HW], F32)
        sav_hb = state.tile([P, HW], F16)
        sav_cb = state.tile([P, HW], F32)

        def barrier():
            tc.strict_bb_all_engine_barrier()

        def store_wide(wxsb, xg_dram, nt):
            """One DMA for a whole token tile of xg (all 16 gate chunks)."""
            dst = xg_dram[:, ds(nt * NB, NB), :]
            nc.sync.dma_start(dst, wxsb[:].rearrange("p (r c) -> p r c", c=U * XW))

        def wx_view(wxsb, pm):
            """[P, r, u, b] view of the wide xg staging for gate chunk pm
            (dram col within a block row = u*XW + pm*BL + b)."""
            v = wxsb[:].rearrange("p (r u m b) -> p r u m b", u=U, m=NM, b=BL)
            return v[:, :, :, pm, :]

        def ps_tok_view(ps):
            return ps[:].rearrange("p (r u b) -> p r u b", u=U, b=BL)

        def onehot_gemm(tab_sb, xg_dram):
            """xg[t] = table[x[t]] via one-hot GEMM. table includes bias.
            Gate chunks in PERM (f,i,g,o) order."""
            for nt in range(NTILES):
                rhs = oh_sb[:, nt * NTT : (nt + 1) * NTT]
                wxsb = wxpool.tile([P, NB * U * XW], F16, tag="wx", name="wx")
                for pm in range(NM):
                    m = PERM[pm]
                    ps = pspool.tile([P, NTT], F32)
                    nc.tensor.matmul(
                        ps[:], tab_sb[:, m * P : (m + 1) * P], rhs, start=True, stop=True
                    )
                    nc.scalar.activation(wx_view(wxsb, pm), ps_tok_view(ps), AF.Identity)
                store_wide(wxsb, xg_dram, nt)

        def load_y_block(src_dram, nt):
            """[P, TT*HW] tile: whole y token tile (all hidden chunks)."""
            yb = ypool.tile([P, TT * HW], F16, name="yblk")
            nc.sync.dma_start(
                yb[:].rearrange("p (r c) -> p r c", c=U * HW),
                src_dram[:, ds(nt * NB, NB), :],
            )
            return yb

        def y_rhs(yb, k):
            """moving operand [P, TT, BL] = hidden chunk k of a y block tile."""
            return yb[:].rearrange("p (t k b) -> p t k b", k=NK, b=BL)[:, :, k, :]

        def proj_tile(wih_sb, ybs, bias_tile, xg_dram, nt, extra_first=None):
            """One token tile of xg = sum_k WihT[k] @ y_k (+ onehot term) + bias.
            ybs: list of y block tiles (each contributes NK contraction chunks)."""
            wxsb = wxpool.tile([P, NB * U * XW], F16, tag="wx", name="wx")
            for pm in range(NM):
                m = PERM[pm]
                ps = pspool.tile([P, NTT], F32)
                first = True
                if extra_first is not None:
                    nc.tensor.matmul(
                        ps[:],
                        extra_first[:, m * P : (m + 1) * P],
                        oh_sb[:, nt * NTT : (nt + 1) * NTT],
                        start=True,
                        stop=False,
                    )
                    first = False
                nyb = len(ybs)
                for yi, yb in enumerate(ybs):
                    for k in range(NK):
                        nc.tensor.matmul(
                            ps[:],
                            wih_sb[:, (yi * NK + k) * G + pm * P : (yi * NK + k) * G + (pm + 1) * P],
                            y_rhs(yb, k),
                            start=first,
                            stop=(yi == nyb - 1 and k == NK - 1),
                        )
                        first = False
                if bias_tile is not None:
                    nc.scalar.activation(
                        wx_view(wxsb, pm), ps_tok_view(ps), AF.Identity,
                        bias=bias_tile[:, m : m + 1],
                    )
                else:
                    nc.scalar.activation(wx_view(wxsb, pm), ps_tok_view(ps), AF.Identity)
            store_wide(wxsb, xg_dram, nt)

        def load_whh(src, tag):
            w = whpool.tile([P, NK * G], WHH_DT, tag=tag, name=tag)
            nc.sync.dma_start(w[:], src[:])
            return w

        # ---------------- fused pair scan ----------------

        def pair_half(whhX, whhY, xgX, xgY, ybX, ybY, blk, half,
                      rev_y, lagb, do_x, do_y):
            """U scan steps for both scans of a pair, staged in st[half].

            blk: For_i expr for this half's staging-block index (of scan X).
            rev_y: encoder pairs — Y is the backward scan (h slots reversed,
                   xg/y blocks at TB-1-blk). lagb: decoder — Y reads xg block
                   blk-lagb and stores y block blk-lagb.
            do_x/do_y: load real xg + store y for that scan (else zs feeds
                   the identity matmul and the y store is skipped).
            """
            stC, stP = st[half], st[1 - half]
            if do_x:
                xgX_sb = xgpool.tile([P, U * XW], F16, tag="xgX", bufs=2,
                                     name="xgX_sb")
                nc.sync.dma_start(
                    xgX_sb[:].rearrange("p (o c) -> p o c", o=1),
                    xgX[:, ds(blk, 1), :])
            if do_y:
                yblk = (TB - 1) - blk if rev_y else blk - lagb
                xgY_sb = xgpool.tile([P, U * XW], F16, tag="xgY", bufs=2,
                                     name="xgY_sb")
                nc.sync.dma_start(
                    xgY_sb[:].rearrange("p (o c) -> p o c", o=1),
                    xgY[:, ds(yblk, 1), :])

            for u in range(U):
                # X gates in psum bank 0 (cols 0:128), Y in bank 1 (cols
                # 512:640): each scan's start=True identity matmul owns its
                # own 2KB zero region.
                ps = ps2pool.tile([P, 1024], F32, tag="pps", name="pps", bufs=2)
                mvX = xgX_sb[:, u * XW : (u + 1) * XW] if do_x else zs[:]
                if do_y:
                    uy = (U - 1 - u) if rev_y else u
                    mvY = xgY_sb[:, uy * XW : (uy + 1) * XW]
                else:
                    mvY = zs[:]
                nc.tensor.matmul(ps[:, 0:128], ident_sb[:], mvX, start=True, stop=False)
                nc.tensor.matmul(ps[:, SOFF : SOFF + 128], ident_sb[:], mvY,
                                 start=True, stop=False)

                # h source slots for this step
                def h_ap(s):
                    if u == 0:
                        slot = (U - 1) if (s == 0 or not rev_y) else 0
                        src = stP
                    else:
                        slot = (u - 1) if (s == 0 or not rev_y) else (U - u)
                        src = stC
                    off = s * YO + slot * HW
                    return src[:, off : off + HW]

                hX, hY = h_ap(0), h_ap(1)
                # weight matmuls: f block first, then i, g, o
                for gate in (0, 1, 2, 3):
                    for s in (0, 1):
                        wsb = whhX if s == 0 else whhY
                        hsrc = hX if s == 0 else hY
                        for j in range(4):
                            pm = gate * 4 + j
                            col = s * SOFF + gate * 32 + j * 8
                            for k in range(NK):
                                # stop only on the final matmul of this
                                # scan's zero region (group is per 2KB bank)
                                nc.tensor.matmul(
                                    ps[:, col : col + 8],
                                    wsb[:, k * G + pm * P : k * G + (pm + 1) * P],
                                    hsrc[:, k * 8 : (k + 1) * 8],
                                    start=False,
                                    stop=(gate == 3 and j == 3 and k == NK - 1),
                                )

                ps3 = ps[:].rearrange("p (s c) -> p s c", c=SOFF)
                sf = spool.tile([P, 64], F16, tag="sf", name="sf", bufs=2)
                sig = spool.tile([P, 128], F16, tag="sig", name="sig", bufs=2)
                tg = spool.tile([P, 64], F16, tag="tg", name="tg", bufs=2)
                so = spool.tile([P, 64], F16, tag="so", name="so", bufs=2)
                th = spool.tile([P, 64], F16, tag="th", name="th", bufs=2)
                t1 = spool.tile([P, 64], F16, tag="t1", name="t1", bufs=2)

                c3 = cT[:].rearrange("p (s c) -> p s c", c=32)
                sf3 = sf[:].rearrange("p (s c) -> p s c", c=32)
                # sigmoid(f) as soon as the f matmuls land, c *= sig_f
                nc.scalar.activation(sf3, ps3[:, :, 0:32], AF.Sigmoid, scale=1.0 / WSCALE)
                nc.vector.tensor_tensor(c3, sf3, c3, ALU.mult)
                # sigmoid over (i|g) (g pre-doubled: tanh(g) = 2*sig(2g)-1)
                sig3 = sig[:].rearrange("p (s c) -> p s c", c=64)
                nc.scalar.activation(sig3, ps3[:, :, 32:96], AF.Sigmoid, scale=1.0 / WSCALE)
                sig4 = sig[:].rearrange("p (s g c) -> p s g c", g=2, c=32)
                tg3 = tg[:].rearrange("p (s c) -> p s c", c=32)
                nc.vector.tensor_scalar(tg3, sig4[:, :, 1, :], 2.0, -1.0,
                                        ALU.mult, ALU.add)
                t13 = t1[:].rearrange("p (s c) -> p s c", c=32)
                nc.vector.tensor_tensor(t13, sig4[:, :, 0, :], tg3, ALU.mult)
                nc.vector.tensor_tensor(cT[:], cT[:], t1[:], ALU.add)
                # sig(o) off the critical path; tanh(c) back on it
                so3 = so[:].rearrange("p (s c) -> p s c", c=32)
                nc.scalar.activation(so3, ps3[:, :, 96:128], AF.Sigmoid, scale=1.0 / WSCALE)
                nc.scalar.activation(th[:], cT[:], AF.Tanh)
                slotY = (U - 1 - u) if rev_y else u
                nc.vector.tensor_tensor(
                    stC[:, u * HW : (u + 1) * HW], so[:, 0:HW], th[:, 0:HW], ALU.mult
                )
                nc.vector.tensor_tensor(
                    stC[:, YO + slotY * HW : YO + (slotY + 1) * HW],
                    so[:, HW : 2 * HW], th[:, HW : 2 * HW], ALU.mult,
                )

            if do_x:
                nc.sync.dma_start(
                    ybX[:, ds(blk, 1), :],
                    stC[:, 0:YO].rearrange("p (o c) -> p o c", o=1))
            if do_y:
                yblk = (TB - 1) - blk if rev_y else blk - lagb
                nc.sync.dma_start(
                    ybY[:, ds(yblk, 1), :],
                    stC[:, YO : 2 * YO].rearrange("p (o c) -> p o c", o=1))

        def pair_loop(lo, hi, whhX, whhY, xgX, xgY, ybX, ybY,
                      rev_y=False, lagb=0, do_x=True, do_y=True):
            with tc.For_i(lo // U, hi // U, 2,
                          hint_engines=(mybir.EngineType.PE,)) as iv:
                for half in range(2):
                    pair_half(whhX, whhY, xgX, xgY, ybX, ybY,
                              iv + half, half, rev_y, lagb, do_x, do_y)

        def init_zero():
            nc.vector.memset(st[1][:], 0.0)
            nc.vector.memset(cT[:], 0.0)

        # ---- phase 1: layer-0 input projections (table gathers) ----
        tabf_sb = wpool.tile([P, G], F16, tag="tab")
        nc.sync.dma_start(tabf_sb[:], tab_f[:])
        tabb_sb = wpool.tile([P, G], F16, tag="tab2")
        nc.sync.dma_start(tabb_sb[:], tab_b[:])
        onehot_gemm(tabf_sb, xg["af"])
        onehot_gemm(tabb_sb, xg["ab"])
        whf = load_whh(whh["l0f"], "whhX")
        whb = load_whh(whh["l0b"], "whhY")
        init_zero()
        barrier()

        # ---- layer-0 scans (fused fwd/bwd) ----
        pair_loop(0, T, whf, whb, xg["af"], xg["ab"], ybuf["l0f"], ybuf["l0b"],
                  rev_y=True)
        barrier()
        # save l0 final states for decoder init: X last h at st[1] slot U-1,
        # Y (reversed slots) last h at st[1] slot 0
        nc.vector.tensor_copy(sav_hf[:], st[1][:, (U - 1) * HW : U * HW])
        nc.vector.tensor_copy(sav_cf[:], cT[:, 0:HW])
        nc.vector.tensor_copy(sav_hb[:], st[1][:, YO : YO + HW])
        nc.vector.tensor_copy(sav_cb[:], cT[:, HW : 2 * HW])

        # ---- layer-1 input projections ----
        wf_sb = wpool.tile([P, 8 * G], F16, tag="wih")
        nc.sync.dma_start(wf_sb[:], wih_l1f[:])
        wb_sb = wpool.tile([P, 8 * G], F16, tag="wih2")
        nc.sync.dma_start(wb_sb[:], wih_l1b[:])

        for nt in range(NTILES):
            ybs = [load_y_block(ybuf["l0f"], nt), load_y_block(ybuf["l0b"], nt)]
            proj_tile(wf_sb, ybs, bias_sb["l1f"], xg["bf"], nt)
            proj_tile(wb_sb, ybs, bias_sb["l1b"], xg["bb"], nt)
        whf = load_whh(whh["l1f"], "whhX")
        whb = load_whh(whh["l1b"], "whhY")
        init_zero()
        barrier()

        # ---- layer-1 scans (fused fwd/bwd) ----
        pair_loop(0, T, whf, whb, xg["bf"], xg["bb"], ybuf["l1f"], ybuf["l1b"],
                  rev_y=True)
        barrier()

        # ---- decoder layer-0 input projection (emb table + enc_out GEMM) ----
        wd_sb = wpool.tile([P, 8 * G], F16, tag="wih")
        nc.sync.dma_start(wd_sb[:], wih_d0e[:])
        tabd_sb = wpool.tile([P, G], F16, tag="tab")
        nc.sync.dma_start(tabd_sb[:], tab_d[:])

        for nt in range(NTILES):
            ybs = [load_y_block(ybuf["l1f"], nt), load_y_block(ybuf["l1b"], nt)]
            proj_tile(wd_sb, ybs, None, xg["af"], nt, extra_first=tabd_sb)
        wh0 = load_whh(whh["d0"], "whhX")
        wh1 = load_whh(whh["d1"], "whhY")
        wd1_sb = wpool.tile([P, 4 * G], F16, tag="wih2")
        nc.sync.dma_start(wd1_sb[:], wih_d1[:])
        # d0 init = l0f final state; d1 runs zeroed until its init below
        init_zero()
        nc.vector.tensor_copy(st[1][:, (U - 1) * HW : U * HW], sav_hf[:])
        nc.vector.tensor_copy(cT[:, 0:HW], sav_cf[:])
        barrier()

        def d1_proj_chunk(k):
            ybs = [load_y_block(ybuf["d0"], k)]
            proj_tile(wd1_sb, ybs, bias_sb["d1"], xg["bf"], k)

        # decoder: d0 at step t fused with d1 at step t-CH (xg["af"] drives d0,
        # xg["bf"] drives d1). head: d0 only; tail: d1 only.
        pair_loop(0, CH, wh0, wh1, xg["af"], xg["bf"], ybuf["d0"], ybuf["d1"],
                  lagb=CHB, do_y=False)
        barrier()
        # d1 init = l0b final state (overwrite the head's zero-run state)
        nc.vector.tensor_copy(st[1][:, YO + (U - 1) * HW : YO + U * HW], sav_hb[:])
        nc.vector.tensor_copy(cT[:, HW : 2 * HW], sav_cb[:])
        d1_proj_chunk(0)
        barrier()
        for kc in range(1, NCH):
            pair_loop(kc * CH, (kc + 1) * CH, wh0, wh1, xg["af"], xg["bf"],
                      ybuf["d0"], ybuf["d1"], lagb=CHB)
            barrier()
            d1_proj_chunk(kc)
            barrier()
        pair_loop(T, T + CH, wh0, wh1, xg["af"], xg["bf"], ybuf["d0"],
                  ybuf["d1"], lagb=CHB, do_x=False)
        barrier()

        # ---- output projection ----
        ow_sb = wpool.tile([P, NK * V], F16, tag="tab")
        nc.sync.dma_start(ow_sb[:], owT[:])
        for nt in range(NTILES):
            yb = load_y_block(ybuf["d1"], nt)
            ps = pspool.tile([P, NTT], F32)
            for k in range(NK):
                nc.tensor.matmul(
                    ps[:],
                    ow_sb[:, k * V : (k + 1) * V],
                    y_rhs(yb, k),
                    start=(k == 0),
                    stop=(k == NK - 1),
                )
            xsb = xsbpool.tile([P, NTT], F32)
            nc.scalar.activation(xsb[:], ps[:], AF.Identity, bias=bout_sb[:])
            nc.sync.dma_start(logitsT[:, nt * NTT : (nt + 1) * NTT], xsb[:])

    nc.finalize()
    return nc


# ---------------- host-side packing ----------------

_G_SCALE = np.ones(NM, np.float64)
_G_SCALE[8:12] = 2.0  # g-gate rows doubled: tanh(g) == 2*sigmoid(2g)-1


def _whh_np_dtype():
    return mybir.dt.np(WHH_DT)


def _pack_whhT(Whh):
    """[P, NK*G]; column block (k, pm) holds Whh[PERM[pm]-chunk, k-chunk].T"""
    out = np.empty((P, NK * G), _whh_np_dtype())
    for k in range(NK):
        for pm in range(NM):
            m = PERM[pm]
            out[:, k * G + pm * P : k * G + (pm + 1) * P] = (
                Whh[m * P : (m + 1) * P, k * P : (k + 1) * P].T * (_G_SCALE[m] * WSCALE)
            ).astype(_whh_np_dtype())
    return out


def _pack_wihT(Wih, col_off, nkc):
    out = np.empty((P, nkc * G), np.float16)
    for k in range(nkc):
        c = col_off + k * P
        for pm in range(NM):
            m = PERM[pm]
            out[:, k * G + pm * P : k * G + (pm + 1) * P] = (
                Wih[m * P : (m + 1) * P, c : c + P].T * (_G_SCALE[m] * WSCALE)
            ).astype(np.float16)
    return out


def _pack_table(emb, Wih_sub, bias):
    tab = emb.astype(np.float64) @ Wih_sub.astype(np.float64).T + bias.astype(np.float64)
    tab = tab * (np.repeat(_G_SCALE, P)[None, :] * WSCALE)  # g doubling + WSCALE
    return tab.astype(np.float16)  # [V, G], original m order (device applies PERM)


def _pack_bias(bih, bhh):
    b = (bih + bhh).astype(np.float64) * np.repeat(_G_SCALE, P) * WSCALE
    return b.reshape(NM, P).T.astype(np.float32).copy()  # [p, m] (original m)


_CACHE = {}
LAST_EXEC_NS = None
LAST_RAW_NS = None


def _run_spmd_timed(nc, in_maps, iters=3):
    """Mirror run_bass_via_pjrt's multi-core path, but device_put inputs once
    so repeated executions time (exec + dispatch), not input upload."""
    import time as _time

    import jax
    import jax.numpy as jnp
    import concourse.mybir as mybir_
    from concourse import bass2jax
    from jax.experimental.shard_map import shard_map
    from jax.sharding import Mesh, NamedSharding, PartitionSpec

    bass2jax.install_neuronx_cc_hook()
    n_cores = len(in_maps)
    partition_name = nc.partition_id_tensor.name if nc.partition_id_tensor else None

    in_names, out_names, out_avals, zero_outs = [], [], [], []
    for alloc in nc.m.functions[0].allocations:
        if not isinstance(alloc, mybir_.MemoryLocationSet):
            continue
        name = alloc.memorylocations[0].name
        if alloc.kind == "ExternalInput":
            if name != partition_name:
                in_names.append(name)
        elif alloc.kind == "ExternalOutput":
            out_names.append(name)
            shape = tuple(alloc.tensor_shape)
            dtype = mybir_.dt.np(alloc.dtype)
            out_avals.append(jax.core.ShapedArray(shape, dtype))
            zero_outs.append(np.zeros(shape, dtype))
    n_params = len(in_names)
    n_outs = len(out_avals)
    all_in_names = list(in_names) + list(out_names)
    if partition_name is not None:
        all_in_names.append(partition_name)

    donate = tuple(range(n_params, n_params + n_outs))

    def _body(*args):
        operands = list(args)
        if partition_name is not None:
            operands.append(bass2jax.partition_id_tensor())
        outs = bass2jax._bass_exec_p.bind(
            *operands,
            out_avals=tuple(out_avals),
            in_names=tuple(all_in_names),
            out_names=tuple(out_names),
            lowering_input_output_aliases=(),
            sim_require_finite=True,
            sim_require_nnan=True,
            nc=nc,
        )
        return tuple(outs)

    devices = jax.devices()[:n_cores]
    mesh = Mesh(np.asarray(devices), ("core",))
    in_specs = (PartitionSpec("core"),) * (n_params + n_outs)
    out_specs = (PartitionSpec("core"),) * len(out_names)
    sharded = jax.jit(
        shard_map(_body, mesh=mesh, in_specs=in_specs, out_specs=out_specs, check_rep=False),
        donate_argnums=donate,
        keep_unused=True,
    )
    shd = NamedSharding(mesh, PartitionSpec("core"))
    concat_in = [
        jax.device_put(
            np.concatenate([np.asarray(in_maps[c][nm]) for c in range(n_cores)], axis=0),
            shd,
        )
        for nm in in_names
    ]
    big_zeros = [np.concatenate([z] * n_cores, axis=0) for z in zero_outs]

    best = None
    out_arrs = None
    for _ in range(max(1, iters)):
        zo = [jax.device_put(z, shd) for z in big_zeros]
        jax.block_until_ready(zo)
        jax.block_until_ready(concat_in)
        t0 = _time.perf_counter()
        out_arrs = sharded(*concat_in, *zo)
        jax.block_until_ready(out_arrs)
        dt = _time.perf_counter() - t0
        best = dt if best is None else min(best, dt)

    _LAST_RUN.clear()
    _LAST_RUN.update(
        sharded=sharded, concat_in=concat_in, big_zeros=big_zeros, shd=shd
    )

    results = []
    for c in range(n_cores):
        d = {}
        for i, nm in enumerate(out_names):
            full = np.asarray(out_arrs[i])
            per = full.shape[0] // n_cores
            d[nm] = full[c * per : (c + 1) * per]
        results.append(d)
    return results, best


_LAST_RUN = {}


def measure_exec_ns(m_lo=4, m_hi=8, reps=3):
    """Slope-based per-exec time: wall(M back-to-back launches) is
    overhead + M*exec, so the marginal cost between M=m_lo and M=m_hi
    cancels the (noisy) per-launch dispatch constant."""
    import time as _time
    import jax

    if not _LAST_RUN:
        return None
    sharded = _LAST_RUN["sharded"]
    concat_in = _LAST_RUN["concat_in"]
    big_zeros = _LAST_RUN["big_zeros"]
    shd = _LAST_RUN["shd"]
    best = {}
    for _ in range(reps):
        for M in (m_lo, m_hi):
            zos = [[jax.device_put(z, shd) for z in big_zeros] for _ in range(M)]
            for zo in zos:
                jax.block_until_ready(zo)
            t0 = _time.perf_counter()
            outs = [sharded(*concat_in, *zo) for zo in zos]
            jax.block_until_ready(outs)
            dt = _time.perf_counter() - t0
            if M not in best or dt < best[M]:
                best[M] = dt
    slope = (best[m_hi] - best[m_lo]) / (m_hi - m_lo)
    return int(slope * 1e9)


def _build_tiny():
    """Trivial kernel used to calibrate per-dispatch overhead."""
    nc = bacc.Bacc(None, target_bir_lowering=False)
    a = nc.dram_tensor("a", [P, P], F32, kind="ExternalInput")
    o = nc.dram_tensor("o", [P, P], F32, kind="ExternalOutput")
    with tile.TileContext(nc) as tc, ExitStack() as ctx:
        pool = ctx.enter_context(tc.tile_pool(name="p", bufs=1))
        t = pool.tile([P, P], F32)
        nc.sync.dma_start(t[:], a[:])
        nc.sync.dma_start(o[:], t[:])
    nc.finalize()
    return nc


def dispatch_baseline_ns(iters=5):
    nc = _CACHE.get("tiny")
    if nc is None:
        nc = _CACHE["tiny"] = _build_tiny()
    a = np.zeros((P, P), np.float32)
    _, best = _run_spmd_timed(nc, [{"a": a}] * NCORES, iters=iters)
    return int(best * 1e9)


def make_inputs(inp):
    emb = inp["emb"].astype(np.float32)
    common = {
        "tab_f": _pack_table(emb, inp["enc_Wih_l0f"], inp["enc_bih_l0f"] + inp["enc_bhh_l0f"]),
        "tab_b": _pack_table(emb, inp["enc_Wih_l0b"], inp["enc_bih_l0b"] + inp["enc_bhh_l0b"]),
        "tab_d": _pack_table(
            emb, inp["dec_Wih_l0"][:, :E], inp["dec_bih_l0"] + inp["dec_bhh_l0"]
        ),
        "whhT_l0f": _pack_whhT(inp["enc_Whh_l0f"]),
        "whhT_l0b": _pack_whhT(inp["enc_Whh_l0b"]),
        "whhT_l1f": _pack_whhT(inp["enc_Whh_l1f"]),
        "whhT_l1b": _pack_whhT(inp["enc_Whh_l1b"]),
        "whhT_d0": _pack_whhT(inp["dec_Whh_l0"]),
        "whhT_d1": _pack_whhT(inp["dec_Whh_l1"]),
        "wihT_l1f": _pack_wihT(inp["enc_Wih_l1f"], 0, 8),
        "wihT_l1b": _pack_wihT(inp["enc_Wih_l1b"], 0, 8),
        "wihT_d0e": _pack_wihT(inp["dec_Wih_l0"], E, 8),
        "wihT_d1": _pack_wihT(inp["dec_Wih_l1"], 0, NK),
        "owT": np.concatenate(
            [inp["out_W"][:, k * P : (k + 1) * P].T for k in range(NK)], axis=1
        ).astype(np.float16),
        "bias_l1f": _pack_bias(inp["enc_bih_l1f"], inp["enc_bhh_l1f"]),
        "bias_l1b": _pack_bias(inp["enc_bih_l1b"], inp["enc_bhh_l1b"]),
        "bias_d1": _pack_bias(inp["dec_bih_l1"], inp["dec_bhh_l1"]),
        "bias_out": inp["out_b"].astype(np.float32).reshape(P, 1),
        "ident": np.eye(P, dtype=np.float16),
    }

    x = np.asarray(inp["x"])
    T = x.shape[1]
    TOK = BL * T
    in_maps = []
    for c in range(NCORES):
        xl = x[c * BL : (c + 1) * BL].astype(np.int64)  # [BL, T]
        oh = np.zeros((V, TOK), np.float16)
        oh[xl.T.reshape(-1), np.arange(TOK)] = 1.0  # col j = t*BL+b
        in_maps.append({**common, "onehotT": oh})
    return in_maps


def kernel(**inp):
    x = np.asarray(inp["x"])
    B_, T = x.shape
    assert B_ == B
    TOK = BL * T

    key = T
    if key not in _CACHE:
        nc = bacc.Bacc(None, target_bir_lowering=False)
        build_model(nc, T)
        _CACHE[key] = nc
    nc = _CACHE[key]

    in_maps = make_inputs(inp)
    results, best_s = _run_spmd_timed(nc, in_maps, iters=3)
    global LAST_EXEC_NS
    LAST_EXEC_NS = int(best_s * 1e9)
    global LAST_RAW_NS
    LAST_RAW_NS = int(best_s * 1e9)

    out = np.empty((B, T, V), np.float32)
    for c in range(NCORES):
        lt = results[c]["logitsT"]  # [V, TOK]
        out[c * BL : (c + 1) * BL] = lt.reshape(V, T, BL).transpose(2, 1, 0)
    return out


# revision 4
# speedup vs baseline: 1.0780x; 1.0780x over previous
"""Trainium2 Bass kernel for nn_DiacriticRestorer (2-layer biLSTM encoder +
2-layer LSTM decoder + linear head), data-parallel over batch on 8 NeuronCores.

v2 design (vs baseline): the scan loop is dependency-chain bound, so the two
interleaved scans of each pair are FUSED into shared wide elementwise ops:

 - psum gates tile [P, 1024] f32 spans 2 banks: scan X gates in bank 0
   (cols 0:128), scan Y in bank 1 (cols 512:640) so each scan's start=True
   identity matmul owns its own 2KB zero region. Within a scan, gate blocks
   are ordered (f, i, g, o), each 32 cols = 4 hidden-chunks x 8 batch.
 - xg (with biases) enters psum via ONE identity-stationary matmul per scan;
   Whh chunk matmuls accumulate on top, f block first so sigmoid(f) can
   start while i/g/o matmuls still run.
 - gate math per pair-step (g rows host-pre-doubled so tanh(g)=2*sig(2g)-1):
   sig_f -> c *= sig_f; sig_ig -> tg = 2*sig_g-1, t1 = sig_i*tg, c += t1;
   sig_o (off critical path); th = tanh(c) (real Tanh table, co-resident
   with Sigmoid -> no table reloads); h = sig_o*th.
 - h goes into a rotating staging tile [P, 2*U*HW] (U=8 steps, X block then
   Y block, backward scans write slot U-1-u); xg loads and y stores are one
   DMA per U steps with 2KB contiguous rows (block-major DRAM layouts).
 - projections write all 16 gate-chunks of a token tile into one wide SBUF
   staging tile and store with a single 2KB-element DMA; y is read back one
   whole block-tile per DMA.
 - decoder pair (d0, d1 with a CH-step lag) shares the same body; head/tail
   segments feed the missing scan a zeroed xg tile and skip its y store.
"""

import numpy as np

import concourse.bacc as bacc
import concourse.bass as bass
import concourse.mybir as mybir
import concourse.tile as tile
from concourse.bass import ds
from concourse.bass_utils import run_bass_kernel_spmd
from contextlib import ExitStack

F16 = mybir.dt.float16
F32 = mybir.dt.float32
AF = mybir.ActivationFunctionType
ALU = mybir.AluOpType

V, E, H, B = 128, 256, 512, 64
NCORES = 8
BL = B // NCORES  # 8
G = 4 * H  # 2048
NK = H // 128  # 4  h chunks
NM = G // 128  # 16 gate chunks
P = 128
U = 8           # steps per staging block / per DMA batch
HALVES = 8      # staging blocks per For_i body (amortizes loop barrier)
XW = NM * BL    # 128: xg cols per step per scan
HW = NK * BL    # 32: h cols per scan
SOFF = 512      # psum col offset of scan Y (bank 1)

# gate-chunk order in xg cols / psum blocks: [f(4), i(4), g(4), o(4)].
# PyTorch gate rows are i(m0-3), f(m4-7), g(m8-11), o(m12-15).
PERM = [4, 5, 6, 7, 0, 1, 2, 3, 8, 9, 10, 11, 12, 13, 14, 15]

WHH_DT = F16   # stationary dtype for recurrent weights
WSCALE = 1.0   # gate-path scale: weights/tables/biases packed xWSCALE,
               # sigmoid ACT ops read psum with scale=1/WSCALE (for fp8 range)


def build_model(nc, T):
    TOK = BL * T
    NTT = min(512, TOK)  # tokens per GEMM tile
    NTILES = TOK // NTT
    TT = NTT // BL  # timesteps per GEMM tile
    NB = TT // U    # staging blocks per GEMM tile (8)
    TB = T // U     # total staging blocks
    CH = TT         # decoder chunk = one proj token-tile worth of steps
    CHB = CH // U
    NCH = T // CH

    ein = lambda name, shape, dt=F16: nc.dram_tensor(
        name, shape, dt, kind="ExternalInput"
    )

    onehotT = ein("onehotT", [P, TOK])
    tab_f = ein("tab_f", [P, G])
    tab_b = ein("tab_b", [P, G])
    tab_d = ein("tab_d", [P, G])
    whh = {
        k: ein(f"whhT_{k}", [P, NK * G], WHH_DT)
        for k in ("l0f", "l0b", "l1f", "l1b", "d0", "d1")
    }
    wih_l1f = ein("wihT_l1f", [P, 8 * G])
    wih_l1b = ein("wihT_l1b", [P, 8 * G])
    wih_d0e = ein("wihT_d0e", [P, 8 * G])
    wih_d1 = ein("wihT_d1", [P, 4 * G])
    owT = ein("owT", [P, NK * V])
    ident = ein("ident", [P, P])
    bias_l1f = ein("bias_l1f", [P, NM], F32)
    bias_l1b = ein("bias_l1b", [P, NM], F32)
    bias_d1 = ein("bias_d1", [P, NM], F32)
    bias_out = ein("bias_out", [P, 1], F32)

    logitsT = nc.dram_tensor("logitsT", [P, TOK], F32, kind="ExternalOutput")

    # internal DRAM scratch, block-major: row = U consecutive steps (2KB rows)
    xg = {
        k: nc.dram_tensor(f"xg_{k}", [P, TB, U * XW], F16)
        for k in ("af", "ab", "bf", "bb")
    }
    ybuf = {
        k: nc.dram_tensor(f"y_{k}", [P, TB, U * HW], F16)
        for k in ("l0f", "l0b", "l1f", "l1b", "d0", "d1")
    }

    with tile.TileContext(nc) as tc, ExitStack() as ctx:
        const = ctx.enter_context(tc.tile_pool(name="const", bufs=1))
        wpool = ctx.enter_context(tc.tile_pool(name="wpool", bufs=1))
        whpool = ctx.enter_context(tc.tile_pool(name="whpool", bufs=1))
        ypool = ctx.enter_context(tc.tile_pool(name="ybl", bufs=4))
        pspool = ctx.enter_context(tc.tile_pool(name="ps", bufs=3, space="PSUM"))
        ps2pool = ctx.enter_context(tc.tile_pool(name="ps2", bufs=2, space="PSUM"))
        wxpool = ctx.enter_context(tc.tile_pool(name="wx", bufs=2))
        xsbpool = ctx.enter_context(tc.tile_pool(name="xsb", bufs=4))
        spool = ctx.enter_context(tc.tile_pool(name="scan", bufs=2))
        xgpool = ctx.enter_context(tc.tile_pool(name="xgp", bufs=4))
        state = ctx.enter_context(tc.tile_pool(name="state", bufs=1))

        oh_sb = const.tile([P, TOK], F16)
        nc.sync.dma_start(oh_sb[:], onehotT[:])
        bias_sb = {}
        for nm, t in (("l1f", bias_l1f), ("l1b", bias_l1b), ("d1", bias_d1)):
            bias_sb[nm] = const.tile([P, NM], F32, name=f"bias_{nm}", tag=f"bias_{nm}")
            nc.sync.dma_start(bias_sb[nm][:], t[:])
        bout_sb = const.tile([P, 1], F32)
        nc.sync.dma_start(bout_sb[:], bias_out[:])
        ident_sb = const.tile([P, P], F16)
        nc.sync.dma_start(ident_sb[:], ident[:])
        zs = const.tile([P, XW], F16, name="zs")
        nc.vector.memset(zs[:], 0.0)

        # persistent scan state: two staging buffers (h history) + cell state.
        # staging layout: X block cols [0, U*HW), Y block cols [U*HW, 2*U*HW);
        # within a block, slot w at w*HW.
        YO = U * HW  # 256
        st = [state.tile([P, 2 * U * HW], F16, name=f"st{i}", tag=f"st{i}")
              for i in range(2)]
        cT = state.tile([P, 2 * HW], F32, name="cT")
        sav_hf = state.tile([P, HW], F16)
        sav_cf = state.tile([P, HW], F32)
        sav_hb = state.tile([P, HW], F16)
        sav_cb = state.tile([P, HW], F32)

        def barrier():
            tc.strict_bb_all_engine_barrier()

        def store_wide(wxsb, xg_dram, nt):
            """One DMA for a whole token tile of xg (all 16 gate chunks)."""
            dst = xg_dram[:, ds(nt * NB, NB), :]
            nc.sync.dma_start(dst, wxsb[:].rearrange("p (r c) -> p r c", c=U * XW))

        def wx_view(wxsb, pm):
            """[P, r, u, b] view of the wide xg staging for gate chunk pm
            (dram col within a block row = u*XW + pm*BL + b)."""
            v = wxsb[:].rearrange("p (r u m b) -> p r u m b", u=U, m=NM, b=BL)
            return v[:, :, :, pm, :]

        def ps_tok_view(ps):
            return ps[:].rearrange("p (r u b) -> p r u b", u=U, b=BL)

        def onehot_gemm(tab_sb, xg_dram):
            """xg[t] = table[x[t]] via one-hot GEMM. table includes bias.
            Gate chunks in PERM (f,i,g,o) order."""
            for nt in range(NTILES):
                rhs = oh_sb[:, nt * NTT : (nt + 1) * NTT]
                wxsb = wxpool.tile([P, NB * U * XW], F16, tag="wx", name="wx")
                for pm in range(NM):
                    m = PERM[pm]
                    ps = pspool.tile([P, NTT], F32)
                    nc.tensor.matmul(
                        ps[:], tab_sb[:, m * P : (m + 1) * P], rhs, start=True, stop=True
                    )
                    nc.scalar.activation(wx_view(wxsb, pm), ps_tok_view(ps), AF.Identity)
                store_wide(wxsb, xg_dram, nt)

        def load_y_block(src_dram, nt):
            """[P, TT*HW] tile: whole y token tile (all hidden chunks)."""
            yb = ypool.tile([P, TT * HW], F16, name="yblk")
            nc.sync.dma_start(
                yb[:].rearrange("p (r c) -> p r c", c=U * HW),
                src_dram[:, ds(nt * NB, NB), :],
            )
            return yb

        def y_rhs(yb, k):
            """moving operand [P, TT, BL] = hidden chunk k of a y block tile."""
            return yb[:].rearrange("p (t k b) -> p t k b", k=NK, b=BL)[:, :, k, :]

        def proj_tile(wih_sb, ybs, bias_tile, xg_dram, nt, extra_first=None):
            """One token tile of xg = sum_k WihT[k] @ y_k (+ onehot term) + bias.
            ybs: list of y block tiles (each contributes NK contraction chunks)."""
            wxsb = wxpool.tile([P, NB * U * XW], F16, tag="wx", name="wx")
            for pm in range(NM):
                m = PERM[pm]
                ps = pspool.tile([P, NTT], F32)
                first = True
                if extra_first is not None:
                    nc.tensor.matmul(
                        ps[:],
                        extra_first[:, m * P : (m + 1) * P],
                        oh_sb[:, nt * NTT : (nt + 1) * NTT],
                        start=True,
                        stop=False,
                    )
                    first = False
                nyb = len(ybs)
                for yi, yb in enumerate(ybs):
                    for k in range(NK):
                        nc.tensor.matmul(
                            ps[:],
                            wih_sb[:, (yi * NK + k) * G + pm * P : (yi * NK + k) * G + (pm + 1) * P],
                            y_rhs(yb, k),
                            start=first,
                            stop=(yi == nyb - 1 and k == NK - 1),
                        )
                        first = False
                if bias_tile is not None:
                    nc.scalar.activation(
                        wx_view(wxsb, pm), ps_tok_view(ps), AF.Identity,
                        bias=bias_tile[:, m : m + 1],
                    )
                else:
                    nc.scalar.activation(wx_view(wxsb, pm), ps_tok_view(ps), AF.Identity)
            store_wide(wxsb, xg_dram, nt)

        def load_whh(src, tag):
            w = whpool.tile([P, NK * G], WHH_DT, tag=tag, name=tag)
            nc.sync.dma_start(w[:], src[:])
            return w

        # ---------------- fused pair scan ----------------

        def pair_half(whhX, whhY, xgX, xgY, ybX, ybY, blk, half,
                      rev_y, lagb, do_x, do_y):
            """U scan steps for both scans of a pair, staged in st[half].

            blk: For_i expr for this half's staging-block index (of scan X).
            rev_y: encoder pairs — Y is the backward scan (h slots reversed,
                   xg/y blocks at TB-1-blk). lagb: decoder — Y reads xg block
                   blk-lagb and stores y block blk-lagb.
            do_x/do_y: load real xg + store y for that scan (else zs feeds
                   the identity matmul and the y store is skipped).
            """
            stC, stP = st[half], st[1 - half]
            if do_x:
                xgX_sb = xgpool.tile([P, U * XW], F16, tag="xgX", bufs=2,
                                     name="xgX_sb")
                nc.sync.dma_start(
                    xgX_sb[:].rearrange("p (o c) -> p o c", o=1),
                    xgX[:, ds(blk, 1), :])
            if do_y:
                yblk = (TB - 1) - blk if rev_y else blk - lagb
                xgY_sb = xgpool.tile([P, U * XW], F16, tag="xgY", bufs=2,
                                     name="xgY_sb")
                nc.sync.dma_start(
                    xgY_sb[:].rearrange("p (o c) -> p o c", o=1),
                    xgY[:, ds(yblk, 1), :])

            for u in range(U):
                # X gates in psum bank 0 (cols 0:128), Y in bank 1 (cols
                # 512:640): each scan's start=True identity matmul owns its
                # own 2KB zero region.
                ps = ps2pool.tile([P, 1024], F32, tag="pps", name="pps", bufs=2)
                mvX = xgX_sb[:, u * XW : (u + 1) * XW] if do_x else zs[:]
                if do_y:
                    uy = (U - 1 - u) if rev_y else u
                    mvY = xgY_sb[:, uy * XW : (uy + 1) * XW]
                else:
                    mvY = zs[:]
                nc.tensor.matmul(ps[:, 0:128], ident_sb[:], mvX, start=True, stop=False)
                nc.tensor.matmul(ps[:, SOFF : SOFF + 128], ident_sb[:], mvY,
                                 start=True, stop=False)

                # h source slots for this step
                def h_ap(s):
                    if u == 0:
                        slot = (U - 1) if (s == 0 or not rev_y) else 0
                        src = stP
                    else:
                        slot = (u - 1) if (s == 0 or not rev_y) else (U - u)
                        src = stC
                    off = s * YO + slot * HW
                    return src[:, off : off + HW]

                hX, hY = h_ap(0), h_ap(1)
                # weight matmuls: f block first, then i, g, o
                for gate in (0, 1, 2, 3):
                    for s in (0, 1):
                        wsb = whhX if s == 0 else whhY
                        hsrc = hX if s == 0 else hY
                        for j in range(4):
                            pm = gate * 4 + j
                            col = s * SOFF + gate * 32 + j * 8
                            for k in range(NK):
                                # stop only on the final matmul of this
                                # scan's zero region (group is per 2KB bank)
                                nc.tensor.matmul(
                                    ps[:, col : col + 8],
                                    wsb[:, k * G + pm * P : k * G + (pm + 1) * P],
                                    hsrc[:, k * 8 : (k + 1) * 8],
                                    start=False,
                                    stop=(gate == 3 and j == 3 and k == NK - 1),
                                )

                ps3 = ps[:].rearrange("p (s c) -> p s c", c=SOFF)
                sf = spool.tile([P, 64], F16, tag="sf", name="sf", bufs=2)
                sig = spool.tile([P, 128], F16, tag="sig", name="sig", bufs=2)
                tg = spool.tile([P, 64], F16, tag="tg", name="tg", bufs=2)
                so = spool.tile([P, 64], F16, tag="so", name="so", bufs=2)
                th = spool.tile([P, 64], F16, tag="th", name="th", bufs=2)
                t1 = spool.tile([P, 64], F16, tag="t1", name="t1", bufs=2)

                c3 = cT[:].rearrange("p (s c) -> p s c", c=32)
                sf3 = sf[:].rearrange("p (s c) -> p s c", c=32)
                # sigmoid(f) as soon as the f matmuls land, c *= sig_f
                nc.scalar.activation(sf3, ps3[:, :, 0:32], AF.Sigmoid, scale=1.0 / WSCALE)
                nc.vector.tensor_tensor(c3, sf3, c3, ALU.mult)
                # sigmoid over (i|g) (g pre-doubled: tanh(g) = 2*sig(2g)-1)
                sig3 = sig[:].rearrange("p (s c) -> p s c", c=64)
                nc.scalar.activation(sig3, ps3[:, :, 32:96], AF.Sigmoid, scale=1.0 / WSCALE)
                sig4 = sig[:].rearrange("p (s g c) -> p s g c", g=2, c=32)
                tg3 = tg[:].rearrange("p (s c) -> p s c", c=32)
                nc.vector.tensor_scalar(tg3, sig4[:, :, 1, :], 2.0, -1.0,
                                        ALU.mult, ALU.add)
                t13 = t1[:].rearrange("p (s c) -> p s c", c=32)
                nc.vector.tensor_tensor(t13, sig4[:, :, 0, :], tg3, ALU.mult)
                nc.vector.tensor_tensor(cT[:], cT[:], t1[:], ALU.add)
                # sig(o) off the critical path; tanh(c) back on it
                so3 = so[:].rearrange("p (s c) -> p s c", c=32)
                nc.scalar.activation(so3, ps3[:, :, 96:128], AF.Sigmoid, scale=1.0 / WSCALE)
                nc.scalar.activation(th[:], cT[:], AF.Tanh)
                slotY = (U - 1 - u) if rev_y else u
                nc.vector.tensor_tensor(
                    stC[:, u * HW : (u + 1) * HW], so[:, 0:HW], th[:, 0:HW], ALU.mult
                )
                nc.vector.tensor_tensor(
                    stC[:, YO + slotY * HW : YO + (slotY + 1) * HW],
                    so[:, HW : 2 * HW], th[:, HW : 2 * HW], ALU.mult,
                )

            if do_x:
                nc.sync.dma_start(
                    ybX[:, ds(blk, 1), :],
                    stC[:, 0:YO].rearrange("p (o c) -> p o c", o=1))
            if do_y:
                yblk = (TB - 1) - blk if rev_y else blk - lagb
                nc.sync.dma_start(
                    ybY[:, ds(yblk, 1), :],
                    stC[:, YO : 2 * YO].rearrange("p (o c) -> p o c", o=1))

        def pair_loop(lo, hi, whhX, whhY, xgX, xgY, ybX, ybY,
                      rev_y=False, lagb=0, do_x=True, do_y=True):
            with tc.For_i(lo // U, hi // U, HALVES,
                          hint_engines=(mybir.EngineType.PE,)) as iv:
                for half in range(HALVES):
                    pair_half(whhX, whhY, xgX, xgY, ybX, ybY,
                              iv + half, half % 2, rev_y, lagb, do_x, do_y)

        def init_zero():
            nc.vector.memset(st[1][:], 0.0)
            nc.vector.memset(cT[:], 0.0)

        # ---- phase 1: layer-0 input projections (table gathers) ----
        tabf_sb = wpool.tile([P, G], F16, tag="tab")
        nc.sync.dma_start(tabf_sb[:], tab_f[:])
        tabb_sb = wpool.tile([P, G], F16, tag="tab2")
        nc.sync.dma_start(tabb_sb[:], tab_b[:])
        onehot_gemm(tabf_sb, xg["af"])
        onehot_gemm(tabb_sb, xg["ab"])
        whf = load_whh(whh["l0f"], "whhX")
        whb = load_whh(whh["l0b"], "whhY")
        init_zero()
        barrier()

        # ---- layer-0 scans (fused fwd/bwd) ----
        pair_loop(0, T, whf, whb, xg["af"], xg["ab"], ybuf["l0f"], ybuf["l0b"],
                  rev_y=True)
        barrier()
        # save l0 final states for decoder init: X last h at st[1] slot U-1,
        # Y (reversed slots) last h at st[1] slot 0
        nc.vector.tensor_copy(sav_hf[:], st[1][:, (U - 1) * HW : U * HW])
        nc.vector.tensor_copy(sav_cf[:], cT[:, 0:HW])
        nc.vector.tensor_copy(sav_hb[:], st[1][:, YO : YO + HW])
        nc.vector.tensor_copy(sav_cb[:], cT[:, HW : 2 * HW])

        # ---- layer-1 input projections ----
        wf_sb = wpool.tile([P, 8 * G], F16, tag="wih")
        nc.sync.dma_start(wf_sb[:], wih_l1f[:])
        wb_sb = wpool.tile([P, 8 * G], F16, tag="wih2")
        nc.sync.dma_start(wb_sb[:], wih_l1b[:])

        for nt in range(NTILES):
            ybs = [load_y_block(ybuf["l0f"], nt), load_y_block(ybuf["l0b"], nt)]
            proj_tile(wf_sb, ybs, bias_sb["l1f"], xg["bf"], nt)
            proj_tile(wb_sb, ybs, bias_sb["l1b"], xg["bb"], nt)
        whf = load_whh(whh["l1f"], "whhX")
        whb = load_whh(whh["l1b"], "whhY")
        init_zero()
        barrier()

        # ---- layer-1 scans (fused fwd/bwd) ----
        pair_loop(0, T, whf, whb, xg["bf"], xg["bb"], ybuf["l1f"], ybuf["l1b"],
                  rev_y=True)
        barrier()

        # ---- decoder layer-0 input projection (emb table + enc_out GEMM) ----
        wd_sb = wpool.tile([P, 8 * G], F16, tag="wih")
        nc.sync.dma_start(wd_sb[:], wih_d0e[:])
        tabd_sb = wpool.tile([P, G], F16, tag="tab")
        nc.sync.dma_start(tabd_sb[:], tab_d[:])

        for nt in range(NTILES):
            ybs = [load_y_block(ybuf["l1f"], nt), load_y_block(ybuf["l1b"], nt)]
            proj_tile(wd_sb, ybs, None, xg["af"], nt, extra_first=tabd_sb)
        wh0 = load_whh(whh["d0"], "whhX")
        wh1 = load_whh(whh["d1"], "whhY")
        wd1_sb = wpool.tile([P, 4 * G], F16, tag="wih2")
        nc.sync.dma_start(wd1_sb[:], wih_d1[:])
        # d0 init = l0f final state; d1 runs zeroed until its init below
        init_zero()
        nc.vector.tensor_copy(st[1][:, (U - 1) * HW : U * HW], sav_hf[:])
        nc.vector.tensor_copy(cT[:, 0:HW], sav_cf[:])
        barrier()

        def d1_proj_chunk(k):
            ybs = [load_y_block(ybuf["d0"], k)]
            proj_tile(wd1_sb, ybs, bias_sb["d1"], xg["bf"], k)

        # decoder: d0 at step t fused with d1 at step t-CH (xg["af"] drives d0,
        # xg["bf"] drives d1). head: d0 only; tail: d1 only.
        pair_loop(0, CH, wh0, wh1, xg["af"], xg["bf"], ybuf["d0"], ybuf["d1"],
                  lagb=CHB, do_y=False)
        barrier()
        # d1 init = l0b final state (overwrite the head's zero-run state)
        nc.vector.tensor_copy(st[1][:, YO + (U - 1) * HW : YO + U * HW], sav_hb[:])
        nc.vector.tensor_copy(cT[:, HW : 2 * HW], sav_cb[:])
        d1_proj_chunk(0)
        barrier()
        for kc in range(1, NCH):
            pair_loop(kc * CH, (kc + 1) * CH, wh0, wh1, xg["af"], xg["bf"],
                      ybuf["d0"], ybuf["d1"], lagb=CHB)
            barrier()
            d1_proj_chunk(kc)
            barrier()
        pair_loop(T, T + CH, wh0, wh1, xg["af"], xg["bf"], ybuf["d0"],
                  ybuf["d1"], lagb=CHB, do_x=False)
        barrier()

        # ---- output projection ----
        ow_sb = wpool.tile([P, NK * V], F16, tag="tab")
        nc.sync.dma_start(ow_sb[:], owT[:])
        for nt in range(NTILES):
            yb = load_y_block(ybuf["d1"], nt)
            ps = pspool.tile([P, NTT], F32)
            for k in range(NK):
                nc.tensor.matmul(
                    ps[:],
                    ow_sb[:, k * V : (k + 1) * V],
                    y_rhs(yb, k),
                    start=(k == 0),
                    stop=(k == NK - 1),
                )
            xsb = xsbpool.tile([P, NTT], F32)
            nc.scalar.activation(xsb[:], ps[:], AF.Identity, bias=bout_sb[:])
            nc.sync.dma_start(logitsT[:, nt * NTT : (nt + 1) * NTT], xsb[:])

    nc.finalize()
    return nc


# ---------------- host-side packing ----------------

_G_SCALE = np.ones(NM, np.float64)
_G_SCALE[8:12] = 2.0  # g-gate rows doubled: tanh(g) == 2*sigmoid(2g)-1


def _whh_np_dtype():
    return mybir.dt.np(WHH_DT)


def _pack_whhT(Whh):
    """[P, NK*G]; column block (k, pm) holds Whh[PERM[pm]-chunk, k-chunk].T"""
    out = np.empty((P, NK * G), _whh_np_dtype())
    for k in range(NK):
        for pm in range(NM):
            m = PERM[pm]
            out[:, k * G + pm * P : k * G + (pm + 1) * P] = (
                Whh[m * P : (m + 1) * P, k * P : (k + 1) * P].T * (_G_SCALE[m] * WSCALE)
            ).astype(_whh_np_dtype())
    return out


def _pack_wihT(Wih, col_off, nkc):
    out = np.empty((P, nkc * G), np.float16)
    for k in range(nkc):
        c = col_off + k * P
        for pm in range(NM):
            m = PERM[pm]
            out[:, k * G + pm * P : k * G + (pm + 1) * P] = (
                Wih[m * P : (m + 1) * P, c : c + P].T * (_G_SCALE[m] * WSCALE)
            ).astype(np.float16)
    return out


def _pack_table(emb, Wih_sub, bias):
    tab = emb.astype(np.float64) @ Wih_sub.astype(np.float64).T + bias.astype(np.float64)
    tab = tab * (np.repeat(_G_SCALE, P)[None, :] * WSCALE)  # g doubling + WSCALE
    return tab.astype(np.float16)  # [V, G], original m order (device applies PERM)


def _pack_bias(bih, bhh):
    b = (bih + bhh).astype(np.float64) * np.repeat(_G_SCALE, P) * WSCALE
    return b.reshape(NM, P).T.astype(np.float32).copy()  # [p, m] (original m)


_CACHE = {}
LAST_EXEC_NS = None
LAST_RAW_NS = None


def _run_spmd_timed(nc, in_maps, iters=3):
    """Mirror run_bass_via_pjrt's multi-core path, but device_put inputs once
    so repeated executions time (exec + dispatch), not input upload."""
    import time as _time

    import jax
    import jax.numpy as jnp
    import concourse.mybir as mybir_
    from concourse import bass2jax
    from jax.experimental.shard_map import shard_map
    from jax.sharding import Mesh, NamedSharding, PartitionSpec

    bass2jax.install_neuronx_cc_hook()
    n_cores = len(in_maps)
    partition_name = nc.partition_id_tensor.name if nc.partition_id_tensor else None

    in_names, out_names, out_avals, zero_outs = [], [], [], []
    for alloc in nc.m.functions[0].allocations:
        if not isinstance(alloc, mybir_.MemoryLocationSet):
            continue
        name = alloc.memorylocations[0].name
        if alloc.kind == "ExternalInput":
            if name != partition_name:
                in_names.append(name)
        elif alloc.kind == "ExternalOutput":
            out_names.append(name)
            shape = tuple(alloc.tensor_shape)
            dtype = mybir_.dt.np(alloc.dtype)
            out_avals.append(jax.core.ShapedArray(shape, dtype))
            zero_outs.append(np.zeros(shape, dtype))
    n_params = len(in_names)
    n_outs = len(out_avals)
    all_in_names = list(in_names) + list(out_names)
    if partition_name is not None:
        all_in_names.append(partition_name)

    donate = tuple(range(n_params, n_params + n_outs))

    def _body(*args):
        operands = list(args)
        if partition_name is not None:
            operands.append(bass2jax.partition_id_tensor())
        outs = bass2jax._bass_exec_p.bind(
            *operands,
            out_avals=tuple(out_avals),
            in_names=tuple(all_in_names),
            out_names=tuple(out_names),
            lowering_input_output_aliases=(),
            sim_require_finite=True,
            sim_require_nnan=True,
            nc=nc,
        )
        return tuple(outs)

    devices = jax.devices()[:n_cores]
    mesh = Mesh(np.asarray(devices), ("core",))
    in_specs = (PartitionSpec("core"),) * (n_params + n_outs)
    out_specs = (PartitionSpec("core"),) * len(out_names)
    sharded = jax.jit(
        shard_map(_body, mesh=mesh, in_specs=in_specs, out_specs=out_specs, check_rep=False),
        donate_argnums=donate,
        keep_unused=True,
    )
    shd = NamedSharding(mesh, PartitionSpec("core"))
    concat_in = [
        jax.device_put(
            np.concatenate([np.asarray(in_maps[c][nm]) for c in range(n_cores)], axis=0),
            shd,
        )
        for nm in in_names
    ]
    big_zeros = [np.concatenate([z] * n_cores, axis=0) for z in zero_outs]

    best = None
    out_arrs = None
    for _ in range(max(1, iters)):
        zo = [jax.device_put(z, shd) for z in big_zeros]
        jax.block_until_ready(zo)
        jax.block_until_ready(concat_in)
        t0 = _time.perf_counter()
        out_arrs = sharded(*concat_in, *zo)
        jax.block_until_ready(out_arrs)
        dt = _time.perf_counter() - t0
        best = dt if best is None else min(best, dt)

    _LAST_RUN.clear()
    _LAST_RUN.update(
        sharded=sharded, concat_in=concat_in, big_zeros=big_zeros, shd=shd
    )

    results = []
    for c in range(n_cores):
        d = {}
        for i, nm in enumerate(out_names):
            full = np.asarray(out_arrs[i])
            per = full.shape[0] // n_cores
            d[nm] = full[c * per : (c + 1) * per]
        results.append(d)
    return results, best


_LAST_RUN = {}


def measure_exec_ns(m_lo=4, m_hi=8, reps=3):
    """Slope-based per-exec time: wall(M back-to-back launches) is
    overhead + M*exec, so the marginal cost between M=m_lo and M=m_hi
    cancels the (noisy) per-launch dispatch constant."""
    import time as _time
    import jax

    if not _LAST_RUN:
        return None
    sharded = _LAST_RUN["sharded"]
    concat_in = _LAST_RUN["concat_in"]
    big_zeros = _LAST_RUN["big_zeros"]
    shd = _LAST_RUN["shd"]
    best = {}
    for _ in range(reps):
        for M in (m_lo, m_hi):
            zos = [[jax.device_put(z, shd) for z in big_zeros] for _ in range(M)]
            for zo in zos:
                jax.block_until_ready(zo)
            t0 = _time.perf_counter()
            outs = [sharded(*concat_in, *zo) for zo in zos]
            jax.block_until_ready(outs)
            dt = _time.perf_counter() - t0
            if M not in best or dt < best[M]:
                best[M] = dt
    slope = (best[m_hi] - best[m_lo]) / (m_hi - m_lo)
    return int(slope * 1e9)


def _build_tiny():
    """Trivial kernel used to calibrate per-dispatch overhead."""
    nc = bacc.Bacc(None, target_bir_lowering=False)
    a = nc.dram_tensor("a", [P, P], F32, kind="ExternalInput")
    o = nc.dram_tensor("o", [P, P], F32, kind="ExternalOutput")
    with tile.TileContext(nc) as tc, ExitStack() as ctx:
        pool = ctx.enter_context(tc.tile_pool(name="p", bufs=1))
        t = pool.tile([P, P], F32)
        nc.sync.dma_start(t[:], a[:])
        nc.sync.dma_start(o[:], t[:])
    nc.finalize()
    return nc


def dispatch_baseline_ns(iters=5):
    nc = _CACHE.get("tiny")
    if nc is None:
        nc = _CACHE["tiny"] = _build_tiny()
    a = np.zeros((P, P), np.float32)
    _, best = _run_spmd_timed(nc, [{"a": a}] * NCORES, iters=iters)
    return int(best * 1e9)


def make_inputs(inp):
    emb = inp["emb"].astype(np.float32)
    common = {
        "tab_f": _pack_table(emb, inp["enc_Wih_l0f"], inp["enc_bih_l0f"] + inp["enc_bhh_l0f"]),
        "tab_b": _pack_table(emb, inp["enc_Wih_l0b"], inp["enc_bih_l0b"] + inp["enc_bhh_l0b"]),
        "tab_d": _pack_table(
            emb, inp["dec_Wih_l0"][:, :E], inp["dec_bih_l0"] + inp["dec_bhh_l0"]
        ),
        "whhT_l0f": _pack_whhT(inp["enc_Whh_l0f"]),
        "whhT_l0b": _pack_whhT(inp["enc_Whh_l0b"]),
        "whhT_l1f": _pack_whhT(inp["enc_Whh_l1f"]),
        "whhT_l1b": _pack_whhT(inp["enc_Whh_l1b"]),
        "whhT_d0": _pack_whhT(inp["dec_Whh_l0"]),
        "whhT_d1": _pack_whhT(inp["dec_Whh_l1"]),
        "wihT_l1f": _pack_wihT(inp["enc_Wih_l1f"], 0, 8),
        "wihT_l1b": _pack_wihT(inp["enc_Wih_l1b"], 0, 8),
        "wihT_d0e": _pack_wihT(inp["dec_Wih_l0"], E, 8),
        "wihT_d1": _pack_wihT(inp["dec_Wih_l1"], 0, NK),
        "owT": np.concatenate(
            [inp["out_W"][:, k * P : (k + 1) * P].T for k in range(NK)], axis=1
        ).astype(np.float16),
        "bias_l1f": _pack_bias(inp["enc_bih_l1f"], inp["enc_bhh_l1f"]),
        "bias_l1b": _pack_bias(inp["enc_bih_l1b"], inp["enc_bhh_l1b"]),
        "bias_d1": _pack_bias(inp["dec_bih_l1"], inp["dec_bhh_l1"]),
        "bias_out": inp["out_b"].astype(np.float32).reshape(P, 1),
        "ident": np.eye(P, dtype=np.float16),
    }

    x = np.asarray(inp["x"])
    T = x.shape[1]
    TOK = BL * T
    in_maps = []
    for c in range(NCORES):
        xl = x[c * BL : (c + 1) * BL].astype(np.int64)  # [BL, T]
        oh = np.zeros((V, TOK), np.float16)
        oh[xl.T.reshape(-1), np.arange(TOK)] = 1.0  # col j = t*BL+b
        in_maps.append({**common, "onehotT": oh})
    return in_maps


def kernel(**inp):
    x = np.asarray(inp["x"])
    B_, T = x.shape
    assert B_ == B
    TOK = BL * T

    key = T
    if key not in _CACHE:
        nc = bacc.Bacc(None, target_bir_lowering=False)
        build_model(nc, T)
        _CACHE[key] = nc
    nc = _CACHE[key]

    in_maps = make_inputs(inp)
    results, best_s = _run_spmd_timed(nc, in_maps, iters=3)
    global LAST_EXEC_NS
    LAST_EXEC_NS = int(best_s * 1e9)
    global LAST_RAW_NS
    LAST_RAW_NS = int(best_s * 1e9)

    out = np.empty((B, T, V), np.float32)
    for c in range(NCORES):
        lt = results[c]["logitsT"]  # [V, TOK]
        out[c * BL : (c + 1) * BL] = lt.reshape(V, T, BL).transpose(2, 1, 0)
    return out


# revision 6
# speedup vs baseline: 1.1088x; 1.0286x over previous
"""Trainium2 Bass kernel for nn_DiacriticRestorer (2-layer biLSTM encoder +
2-layer LSTM decoder + linear head), data-parallel over batch on 8 NeuronCores.

v2 design (vs baseline): the scan loop is dependency-chain bound, so the two
interleaved scans of each pair are FUSED into shared wide elementwise ops:

 - psum gates tile [P, 1024] f32 spans 2 banks: scan X gates in bank 0
   (cols 0:128), scan Y in bank 1 (cols 512:640) so each scan's start=True
   identity matmul owns its own 2KB zero region. Within a scan, gate blocks
   are ordered (f, i, g, o), each 32 cols = 4 hidden-chunks x 8 batch.
 - xg (with biases) enters psum via ONE identity-stationary matmul per scan;
   Whh chunk matmuls accumulate on top, f block first so sigmoid(f) can
   start while i/g/o matmuls still run.
 - gate math per pair-step (g rows host-pre-doubled so tanh(g)=2*sig(2g)-1):
   sig_f -> c *= sig_f; sig_ig -> tg = 2*sig_g-1, t1 = sig_i*tg, c += t1;
   sig_o (off critical path); th = tanh(c) (real Tanh table, co-resident
   with Sigmoid -> no table reloads); h = sig_o*th.
 - h goes into a rotating staging tile [P, 2*U*HW] (U=8 steps, X block then
   Y block, backward scans write slot U-1-u); xg loads and y stores are one
   DMA per U steps with 2KB contiguous rows (block-major DRAM layouts).
 - projections write all 16 gate-chunks of a token tile into one wide SBUF
   staging tile and store with a single 2KB-element DMA; y is read back one
   whole block-tile per DMA.
 - decoder pair (d0, d1 with a CH-step lag) shares the same body; head/tail
   segments feed the missing scan a zeroed xg tile and skip its y store.
"""

import numpy as np

import concourse.bacc as bacc
import concourse.bass as bass
import concourse.mybir as mybir
import concourse.tile as tile
from concourse.bass import ds
from concourse.bass_utils import run_bass_kernel_spmd
from contextlib import ExitStack

F16 = mybir.dt.float16
F32 = mybir.dt.float32
AF = mybir.ActivationFunctionType
ALU = mybir.AluOpType

V, E, H, B = 128, 256, 512, 64
NCORES = 8
BL = B // NCORES  # 8
G = 4 * H  # 2048
NK = H // 128  # 4  h chunks
NM = G // 128  # 16 gate chunks
P = 128
U = 8           # steps per staging block / per DMA batch
HALVES = 8      # staging blocks per For_i body (amortizes loop barrier)
XW = NM * BL    # 128: xg cols per step per scan
HW = NK * BL    # 32: h cols per scan
SOFF = 512      # psum col offset of scan Y (bank 1)

# gate-chunk order in xg cols / psum blocks: [f(4), i(4), g(4), o(4)].
# PyTorch gate rows are i(m0-3), f(m4-7), g(m8-11), o(m12-15).
PERM = [4, 5, 6, 7, 0, 1, 2, 3, 8, 9, 10, 11, 12, 13, 14, 15]

WHH_DT = F16   # stationary dtype for recurrent weights
WSCALE = 1.0   # gate-path scale: weights/tables/biases packed xWSCALE,
               # sigmoid ACT ops read psum with scale=1/WSCALE (for fp8 range)


def build_model(nc, T):
    TOK = BL * T
    NTT = min(512, TOK)  # tokens per GEMM tile
    NTILES = TOK // NTT
    TT = NTT // BL  # timesteps per GEMM tile
    NB = TT // U    # staging blocks per GEMM tile (8)
    TB = T // U     # total staging blocks
    CH = TT         # decoder chunk = one proj token-tile worth of steps
    CHB = CH // U
    NCH = T // CH

    ein = lambda name, shape, dt=F16: nc.dram_tensor(
        name, shape, dt, kind="ExternalInput"
    )

    onehotT = ein("onehotT", [P, TOK])
    tab_f = ein("tab_f", [P, G])
    tab_b = ein("tab_b", [P, G])
    tab_d = ein("tab_d", [P, G])
    whh = {
        k: ein(f"whhT_{k}", [P, NK * G], WHH_DT)
        for k in ("l0f", "l0b", "l1f", "l1b", "d0", "d1")
    }
    wih_l1f = ein("wihT_l1f", [P, 8 * G])
    wih_l1b = ein("wihT_l1b", [P, 8 * G])
    wih_d0e = ein("wihT_d0e", [P, 8 * G])
    wih_d1 = ein("wihT_d1", [P, 4 * G])
    owT = ein("owT", [P, NK * V])
    ident = ein("ident", [P, P])
    bias_l1f = ein("bias_l1f", [P, NM], F32)
    bias_l1b = ein("bias_l1b", [P, NM], F32)
    bias_d1 = ein("bias_d1", [P, NM], F32)
    bias_out = ein("bias_out", [P, 1], F32)

    logitsT = nc.dram_tensor("logitsT", [P, TOK], F32, kind="ExternalOutput")

    # internal DRAM scratch, block-major: row = U consecutive steps (2KB rows)
    xg = {
        k: nc.dram_tensor(f"xg_{k}", [P, TB, U * XW], F16)
        for k in ("af", "ab", "bf", "bb")
    }
    ybuf = {
        k: nc.dram_tensor(f"y_{k}", [P, TB, U * HW], F16)
        for k in ("l0f", "l0b", "l1f", "l1b", "d0", "d1")
    }

    with tile.TileContext(nc) as tc, ExitStack() as ctx:
        const = ctx.enter_context(tc.tile_pool(name="const", bufs=1))
        wpool = ctx.enter_context(tc.tile_pool(name="wpool", bufs=1))
        whpool = ctx.enter_context(tc.tile_pool(name="whpool", bufs=1))
        ypool = ctx.enter_context(tc.tile_pool(name="ybl", bufs=4))
        pspool = ctx.enter_context(tc.tile_pool(name="ps", bufs=3, space="PSUM"))
        ps2pool = ctx.enter_context(tc.tile_pool(name="ps2", bufs=2, space="PSUM"))
        wxpool = ctx.enter_context(tc.tile_pool(name="wx", bufs=2))
        xsbpool = ctx.enter_context(tc.tile_pool(name="xsb", bufs=4))
        spool = ctx.enter_context(tc.tile_pool(name="scan", bufs=2))
        xgpool = ctx.enter_context(tc.tile_pool(name="xgp", bufs=4))
        state = ctx.enter_context(tc.tile_pool(name="state", bufs=1))

        oh_sb = const.tile([P, TOK], F16)
        nc.sync.dma_start(oh_sb[:], onehotT[:])
        bias_sb = {}
        for nm, t in (("l1f", bias_l1f), ("l1b", bias_l1b), ("d1", bias_d1)):
            bias_sb[nm] = const.tile([P, NM], F32, name=f"bias_{nm}", tag=f"bias_{nm}")
            nc.sync.dma_start(bias_sb[nm][:], t[:])
        bout_sb = const.tile([P, 1], F32)
        nc.sync.dma_start(bout_sb[:], bias_out[:])
        ident_sb = const.tile([P, P], F16)
        nc.sync.dma_start(ident_sb[:], ident[:])
        zs = const.tile([P, XW], F16, name="zs")
        nc.vector.memset(zs[:], 0.0)

        # persistent scan state: two staging buffers (h history) + cell state.
        # staging layout: X block cols [0, U*HW), Y block cols [U*HW, 2*U*HW);
        # within a block, slot w at w*HW.
        YO = U * HW  # 256
        st = [state.tile([P, 2 * U * HW], F16, name=f"st{i}", tag=f"st{i}")
              for i in range(2)]
        cT = state.tile([P, 2 * HW], F32, name="cT")
        sav_hf = state.tile([P, HW], F16)
        sav_cf = state.tile([P, HW], F32)
        sav_hb = state.tile([P, HW], F16)
        sav_cb = state.tile([P, HW], F32)

        def barrier():
            tc.strict_bb_all_engine_barrier()

        def store_wide(wxsb, xg_dram, nt):
            """One DMA for a whole token tile of xg (all 16 gate chunks)."""
            dst = xg_dram[:, ds(nt * NB, NB), :]
            nc.sync.dma_start(dst, wxsb[:].rearrange("p (r c) -> p r c", c=U * XW))

        def wx_view(wxsb, pm):
            """[P, r, u, b] view of the wide xg staging for gate chunk pm
            (dram col within a block row = u*XW + pm*BL + b)."""
            v = wxsb[:].rearrange("p (r u m b) -> p r u m b", u=U, m=NM, b=BL)
            return v[:, :, :, pm, :]

        def ps_tok_view(ps):
            return ps[:].rearrange("p (r u b) -> p r u b", u=U, b=BL)

        def onehot_gemm(tab_sb, xg_dram):
            """xg[t] = table[x[t]] via one-hot GEMM. table includes bias.
            Gate chunks in PERM (f,i,g,o) order."""
            for nt in range(NTILES):
                rhs = oh_sb[:, nt * NTT : (nt + 1) * NTT]
                wxsb = wxpool.tile([P, NB * U * XW], F16, tag="wx", name="wx")
                for pm in range(NM):
                    m = PERM[pm]
                    ps = pspool.tile([P, NTT], F32)
                    nc.tensor.matmul(
                        ps[:], tab_sb[:, m * P : (m + 1) * P], rhs, start=True, stop=True
                    )
                    nc.scalar.activation(wx_view(wxsb, pm), ps_tok_view(ps), AF.Identity)
                store_wide(wxsb, xg_dram, nt)

        def load_y_block(src_dram, nt):
            """[P, TT*HW] tile: whole y token tile (all hidden chunks)."""
            yb = ypool.tile([P, TT * HW], F16, name="yblk")
            nc.sync.dma_start(
                yb[:].rearrange("p (r c) -> p r c", c=U * HW),
                src_dram[:, ds(nt * NB, NB), :],
            )
            return yb

        def y_rhs(yb, k):
            """moving operand [P, TT, BL] = hidden chunk k of a y block tile."""
            return yb[:].rearrange("p (t k b) -> p t k b", k=NK, b=BL)[:, :, k, :]

        def proj_tile(wih_sb, ybs, bias_tile, xg_dram, nt, extra_first=None):
            """One token tile of xg = sum_k WihT[k] @ y_k (+ onehot term) + bias.
            ybs: list of y block tiles (each contributes NK contraction chunks)."""
            wxsb = wxpool.tile([P, NB * U * XW], F16, tag="wx", name="wx")
            for pm in range(NM):
                m = PERM[pm]
                ps = pspool.tile([P, NTT], F32)
                first = True
                if extra_first is not None:
                    nc.tensor.matmul(
                        ps[:],
                        extra_first[:, m * P : (m + 1) * P],
                        oh_sb[:, nt * NTT : (nt + 1) * NTT],
                        start=True,
                        stop=False,
                    )
                    first = False
                nyb = len(ybs)
                for yi, yb in enumerate(ybs):
                    for k in range(NK):
                        nc.tensor.matmul(
                            ps[:],
                            wih_sb[:, (yi * NK + k) * G + pm * P : (yi * NK + k) * G + (pm + 1) * P],
                            y_rhs(yb, k),
                            start=first,
                            stop=(yi == nyb - 1 and k == NK - 1),
                        )
                        first = False
                if bias_tile is not None:
                    nc.scalar.activation(
                        wx_view(wxsb, pm), ps_tok_view(ps), AF.Identity,
                        bias=bias_tile[:, m : m + 1],
                    )
                else:
                    nc.scalar.activation(wx_view(wxsb, pm), ps_tok_view(ps), AF.Identity)
            store_wide(wxsb, xg_dram, nt)

        def load_whh(src, tag):
            w = whpool.tile([P, NK * G], WHH_DT, tag=tag, name=tag)
            nc.sync.dma_start(w[:], src[:])
            return w

        # ---------------- fused pair scan ----------------

        def pair_half(whhX, whhY, xgX, xgY, ybX, ybY, blk, half,
                      rev_y, lagb, do_x, do_y):
            """U scan steps for both scans of a pair, staged in st[half].

            blk: For_i expr for this half's staging-block index (of scan X).
            rev_y: encoder pairs — Y is the backward scan (h slots reversed,
                   xg/y blocks at TB-1-blk). lagb: decoder — Y reads xg block
                   blk-lagb and stores y block blk-lagb.
            do_x/do_y: load real xg + store y for that scan (else zs feeds
                   the identity matmul and the y store is skipped).
            """
            stC, stP = st[half], st[1 - half]
            if do_x:
                xgX_sb = xgpool.tile([P, U * XW], F16, tag="xgX", bufs=2,
                                     name="xgX_sb")
                nc.sync.dma_start(
                    xgX_sb[:].rearrange("p (o c) -> p o c", o=1),
                    xgX[:, ds(blk, 1), :])
            if do_y:
                yblk = (TB - 1) - blk if rev_y else blk - lagb
                xgY_sb = xgpool.tile([P, U * XW], F16, tag="xgY", bufs=2,
                                     name="xgY_sb")
                nc.sync.dma_start(
                    xgY_sb[:].rearrange("p (o c) -> p o c", o=1),
                    xgY[:, ds(yblk, 1), :])

            for u in range(U):
                # X gates in psum bank 0 (cols 0:128), Y in bank 1 (cols
                # 512:640): each scan's start=True identity matmul owns its
                # own 2KB zero region.
                ps = ps2pool.tile([P, 1024], F32, tag="pps", name="pps", bufs=2)
                # a missing scan (decoder head/tail) skips its identity and
                # weight matmuls entirely; its elementwise ops read stale
                # psum and produce garbage state that is re-initialized (Y)
                # or never consumed (X) afterwards.
                mvX = xgX_sb[:, u * XW : (u + 1) * XW] if do_x else zs[:]
                nc.tensor.matmul(ps[:, 0:128], ident_sb[:], mvX,
                                 start=True, stop=not do_x)
                if do_y:
                    uy = (U - 1 - u) if rev_y else u
                    mvY = xgY_sb[:, uy * XW : (uy + 1) * XW]
                else:
                    mvY = zs[:]
                nc.tensor.matmul(ps[:, SOFF : SOFF + 128], ident_sb[:], mvY,
                                 start=True, stop=not do_y)

                # h source slots for this step
                def h_ap(s):
                    if u == 0:
                        slot = (U - 1) if (s == 0 or not rev_y) else 0
                        src = stP
                    else:
                        slot = (u - 1) if (s == 0 or not rev_y) else (U - u)
                        src = stC
                    off = s * YO + slot * HW
                    return src[:, off : off + HW]

                hX, hY = h_ap(0), h_ap(1)
                # weight matmuls: f block first, then i, g, o
                for gate in (0, 1, 2, 3):
                    for s in (0, 1):
                        if (s == 0 and not do_x) or (s == 1 and not do_y):
                            continue
                        wsb = whhX if s == 0 else whhY
                        hsrc = hX if s == 0 else hY
                        for j in range(4):
                            pm = gate * 4 + j
                            col = s * SOFF + gate * 32 + j * 8
                            for k in range(NK):
                                # stop only on the final matmul of this
                                # scan's zero region (group is per 2KB bank)
                                nc.tensor.matmul(
                                    ps[:, col : col + 8],
                                    wsb[:, k * G + pm * P : k * G + (pm + 1) * P],
                                    hsrc[:, k * 8 : (k + 1) * 8],
                                    start=False,
                                    stop=(gate == 3 and j == 3 and k == NK - 1),
                                )

                ps3 = ps[:].rearrange("p (s c) -> p s c", c=SOFF)
                sf = spool.tile([P, 64], F16, tag="sf", name="sf", bufs=2)
                sig = spool.tile([P, 128], F16, tag="sig", name="sig", bufs=2)
                tg = spool.tile([P, 64], F16, tag="tg", name="tg", bufs=2)
                so = spool.tile([P, 64], F16, tag="so", name="so", bufs=2)
                th = spool.tile([P, 64], F16, tag="th", name="th", bufs=2)
                t1 = spool.tile([P, 64], F16, tag="t1", name="t1", bufs=2)

                c3 = cT[:].rearrange("p (s c) -> p s c", c=32)
                sf3 = sf[:].rearrange("p (s c) -> p s c", c=32)
                # sigmoid(f) as soon as the f matmuls land, c *= sig_f
                nc.scalar.activation(sf3, ps3[:, :, 0:32], AF.Sigmoid, scale=1.0 / WSCALE)
                nc.vector.tensor_tensor(c3, sf3, c3, ALU.mult)
                # sigmoid over (i|g) (g pre-doubled: tanh(g) = 2*sig(2g)-1)
                sig3 = sig[:].rearrange("p (s c) -> p s c", c=64)
                nc.scalar.activation(sig3, ps3[:, :, 32:96], AF.Sigmoid, scale=1.0 / WSCALE)
                sig4 = sig[:].rearrange("p (s g c) -> p s g c", g=2, c=32)
                tg3 = tg[:].rearrange("p (s c) -> p s c", c=32)
                nc.vector.tensor_scalar(tg3, sig4[:, :, 1, :], 2.0, -1.0,
                                        ALU.mult, ALU.add)
                t13 = t1[:].rearrange("p (s c) -> p s c", c=32)
                nc.vector.tensor_tensor(t13, sig4[:, :, 0, :], tg3, ALU.mult)
                nc.vector.tensor_tensor(cT[:], cT[:], t1[:], ALU.add)
                # sig(o) off the critical path; tanh(c) back on it
                so3 = so[:].rearrange("p (s c) -> p s c", c=32)
                nc.scalar.activation(so3, ps3[:, :, 96:128], AF.Sigmoid, scale=1.0 / WSCALE)
                nc.scalar.activation(th[:], cT[:], AF.Tanh)
                slotY = (U - 1 - u) if rev_y else u
                nc.vector.tensor_tensor(
                    stC[:, u * HW : (u + 1) * HW], so[:, 0:HW], th[:, 0:HW], ALU.mult
                )
                nc.vector.tensor_tensor(
                    stC[:, YO + slotY * HW : YO + (slotY + 1) * HW],
                    so[:, HW : 2 * HW], th[:, HW : 2 * HW], ALU.mult,
                )

            if do_x:
                nc.sync.dma_start(
                    ybX[:, ds(blk, 1), :],
                    stC[:, 0:YO].rearrange("p (o c) -> p o c", o=1))
            if do_y:
                yblk = (TB - 1) - blk if rev_y else blk - lagb
                nc.sync.dma_start(
                    ybY[:, ds(yblk, 1), :],
                    stC[:, YO : 2 * YO].rearrange("p (o c) -> p o c", o=1))

        def pair_loop(lo, hi, whhX, whhY, xgX, xgY, ybX, ybY,
                      rev_y=False, lagb=0, do_x=True, do_y=True):
            with tc.For_i(lo // U, hi // U, HALVES,
                          hint_engines=(mybir.EngineType.PE,)) as iv:
                for half in range(HALVES):
                    pair_half(whhX, whhY, xgX, xgY, ybX, ybY,
                              iv + half, half % 2, rev_y, lagb, do_x, do_y)

        def init_zero():
            nc.vector.memset(st[1][:], 0.0)
            nc.vector.memset(cT[:], 0.0)

        # ---- phase 1: layer-0 input projections (table gathers) ----
        tabf_sb = wpool.tile([P, G], F16, tag="tab")
        nc.sync.dma_start(tabf_sb[:], tab_f[:])
        tabb_sb = wpool.tile([P, G], F16, tag="tab2")
        nc.sync.dma_start(tabb_sb[:], tab_b[:])
        onehot_gemm(tabf_sb, xg["af"])
        onehot_gemm(tabb_sb, xg["ab"])
        whf = load_whh(whh["l0f"], "whhX")
        whb = load_whh(whh["l0b"], "whhY")
        init_zero()
        barrier()

        # ---- layer-0 scans (fused fwd/bwd) ----
        pair_loop(0, T, whf, whb, xg["af"], xg["ab"], ybuf["l0f"], ybuf["l0b"],
                  rev_y=True)
        barrier()
        # save l0 final states for decoder init: X last h at st[1] slot U-1,
        # Y (reversed slots) last h at st[1] slot 0
        nc.vector.tensor_copy(sav_hf[:], st[1][:, (U - 1) * HW : U * HW])
        nc.vector.tensor_copy(sav_cf[:], cT[:, 0:HW])
        nc.vector.tensor_copy(sav_hb[:], st[1][:, YO : YO + HW])
        nc.vector.tensor_copy(sav_cb[:], cT[:, HW : 2 * HW])

        # ---- layer-1 input projections ----
        wf_sb = wpool.tile([P, 8 * G], F16, tag="wih")
        nc.sync.dma_start(wf_sb[:], wih_l1f[:])
        wb_sb = wpool.tile([P, 8 * G], F16, tag="wih2")
        nc.sync.dma_start(wb_sb[:], wih_l1b[:])

        for nt in range(NTILES):
            ybs = [load_y_block(ybuf["l0f"], nt), load_y_block(ybuf["l0b"], nt)]
            proj_tile(wf_sb, ybs, bias_sb["l1f"], xg["bf"], nt)
            proj_tile(wb_sb, ybs, bias_sb["l1b"], xg["bb"], nt)
        whf = load_whh(whh["l1f"], "whhX")
        whb = load_whh(whh["l1b"], "whhY")
        init_zero()
        barrier()

        # ---- layer-1 scans (fused fwd/bwd) ----
        pair_loop(0, T, whf, whb, xg["bf"], xg["bb"], ybuf["l1f"], ybuf["l1b"],
                  rev_y=True)
        barrier()

        # ---- decoder layer-0 input projection (emb table + enc_out GEMM) ----
        wd_sb = wpool.tile([P, 8 * G], F16, tag="wih")
        nc.sync.dma_start(wd_sb[:], wih_d0e[:])
        tabd_sb = wpool.tile([P, G], F16, tag="tab")
        nc.sync.dma_start(tabd_sb[:], tab_d[:])

        for nt in range(NTILES):
            ybs = [load_y_block(ybuf["l1f"], nt), load_y_block(ybuf["l1b"], nt)]
            proj_tile(wd_sb, ybs, None, xg["af"], nt, extra_first=tabd_sb)
        wh0 = load_whh(whh["d0"], "whhX")
        wh1 = load_whh(whh["d1"], "whhY")
        wd1_sb = wpool.tile([P, 4 * G], F16, tag="wih2")
        nc.sync.dma_start(wd1_sb[:], wih_d1[:])
        # d0 init = l0f final state; d1 runs zeroed until its init below
        init_zero()
        nc.vector.tensor_copy(st[1][:, (U - 1) * HW : U * HW], sav_hf[:])
        nc.vector.tensor_copy(cT[:, 0:HW], sav_cf[:])
        barrier()

        def d1_proj_chunk(k):
            ybs = [load_y_block(ybuf["d0"], k)]
            proj_tile(wd1_sb, ybs, bias_sb["d1"], xg["bf"], k)

        # decoder: d0 at step t fused with d1 at step t-CH (xg["af"] drives d0,
        # xg["bf"] drives d1). head: d0 only; tail: d1 only.
        pair_loop(0, CH, wh0, wh1, xg["af"], xg["bf"], ybuf["d0"], ybuf["d1"],
                  lagb=CHB, do_y=False)
        barrier()
        # d1 init = l0b final state (overwrite the head's zero-run state)
        nc.vector.tensor_copy(st[1][:, YO + (U - 1) * HW : YO + U * HW], sav_hb[:])
        nc.vector.tensor_copy(cT[:, HW : 2 * HW], sav_cb[:])
        d1_proj_chunk(0)
        barrier()
        for kc in range(1, NCH):
            pair_loop(kc * CH, (kc + 1) * CH, wh0, wh1, xg["af"], xg["bf"],
                      ybuf["d0"], ybuf["d1"], lagb=CHB)
            barrier()
            d1_proj_chunk(kc)
            barrier()
        pair_loop(T, T + CH, wh0, wh1, xg["af"], xg["bf"], ybuf["d0"],
                  ybuf["d1"], lagb=CHB, do_x=False)
        barrier()

        # ---- output projection ----
        ow_sb = wpool.tile([P, NK * V], F16, tag="tab")
        nc.sync.dma_start(ow_sb[:], owT[:])
        for nt in range(NTILES):
            yb = load_y_block(ybuf["d1"], nt)
            ps = pspool.tile([P, NTT], F32)
            for k in range(NK):
                nc.tensor.matmul(
                    ps[:],
                    ow_sb[:, k * V : (k + 1) * V],
                    y_rhs(yb, k),
                    start=(k == 0),
                    stop=(k == NK - 1),
                )
            xsb = xsbpool.tile([P, NTT], F32)
            nc.scalar.activation(xsb[:], ps[:], AF.Identity, bias=bout_sb[:])
            nc.sync.dma_start(logitsT[:, nt * NTT : (nt + 1) * NTT], xsb[:])

    nc.finalize()
    return nc


# ---------------- host-side packing ----------------

_G_SCALE = np.ones(NM, np.float64)
_G_SCALE[8:12] = 2.0  # g-gate rows doubled: tanh(g) == 2*sigmoid(2g)-1


def _whh_np_dtype():
    return mybir.dt.np(WHH_DT)


def _pack_whhT(Whh):
    """[P, NK*G]; column block (k, pm) holds Whh[PERM[pm]-chunk, k-chunk].T"""
    out = np.empty((P, NK * G), _whh_np_dtype())
    for k in range(NK):
        for pm in range(NM):
            m = PERM[pm]
            out[:, k * G + pm * P : k * G + (pm + 1) * P] = (
                Whh[m * P : (m + 1) * P, k * P : (k + 1) * P].T * (_G_SCALE[m] * WSCALE)
            ).astype(_whh_np_dtype())
    return out


def _pack_wihT(Wih, col_off, nkc):
    out = np.empty((P, nkc * G), np.float16)
    for k in range(nkc):
        c = col_off + k * P
        for pm in range(NM):
            m = PERM[pm]
            out[:, k * G + pm * P : k * G + (pm + 1) * P] = (
                Wih[m * P : (m + 1) * P, c : c + P].T * (_G_SCALE[m] * WSCALE)
            ).astype(np.float16)
    return out


def _pack_table(emb, Wih_sub, bias):
    tab = emb.astype(np.float64) @ Wih_sub.astype(np.float64).T + bias.astype(np.float64)
    tab = tab * (np.repeat(_G_SCALE, P)[None, :] * WSCALE)  # g doubling + WSCALE
    return tab.astype(np.float16)  # [V, G], original m order (device applies PERM)


def _pack_bias(bih, bhh):
    b = (bih + bhh).astype(np.float64) * np.repeat(_G_SCALE, P) * WSCALE
    return b.reshape(NM, P).T.astype(np.float32).copy()  # [p, m] (original m)


_CACHE = {}
LAST_EXEC_NS = None
LAST_RAW_NS = None


def _run_spmd_timed(nc, in_maps, iters=3):
    """Mirror run_bass_via_pjrt's multi-core path, but device_put inputs once
    so repeated executions time (exec + dispatch), not input upload."""
    import time as _time

    import jax
    import jax.numpy as jnp
    import concourse.mybir as mybir_
    from concourse import bass2jax
    from jax.experimental.shard_map import shard_map
    from jax.sharding import Mesh, NamedSharding, PartitionSpec

    bass2jax.install_neuronx_cc_hook()
    n_cores = len(in_maps)
    partition_name = nc.partition_id_tensor.name if nc.partition_id_tensor else None

    in_names, out_names, out_avals, zero_outs = [], [], [], []
    for alloc in nc.m.functions[0].allocations:
        if not isinstance(alloc, mybir_.MemoryLocationSet):
            continue
        name = alloc.memorylocations[0].name
        if alloc.kind == "ExternalInput":
            if name != partition_name:
                in_names.append(name)
        elif alloc.kind == "ExternalOutput":
            out_names.append(name)
            shape = tuple(alloc.tensor_shape)
            dtype = mybir_.dt.np(alloc.dtype)
            out_avals.append(jax.core.ShapedArray(shape, dtype))
            zero_outs.append(np.zeros(shape, dtype))
    n_params = len(in_names)
    n_outs = len(out_avals)
    all_in_names = list(in_names) + list(out_names)
    if partition_name is not None:
        all_in_names.append(partition_name)

    donate = tuple(range(n_params, n_params + n_outs))

    def _body(*args):
        operands = list(args)
        if partition_name is not None:
            operands.append(bass2jax.partition_id_tensor())
        outs = bass2jax._bass_exec_p.bind(
            *operands,
            out_avals=tuple(out_avals),
            in_names=tuple(all_in_names),
            out_names=tuple(out_names),
            lowering_input_output_aliases=(),
            sim_require_finite=True,
            sim_require_nnan=True,
            nc=nc,
        )
        return tuple(outs)

    devices = jax.devices()[:n_cores]
    mesh = Mesh(np.asarray(devices), ("core",))
    in_specs = (PartitionSpec("core"),) * (n_params + n_outs)
    out_specs = (PartitionSpec("core"),) * len(out_names)
    sharded = jax.jit(
        shard_map(_body, mesh=mesh, in_specs=in_specs, out_specs=out_specs, check_rep=False),
        donate_argnums=donate,
        keep_unused=True,
    )
    shd = NamedSharding(mesh, PartitionSpec("core"))
    concat_in = [
        jax.device_put(
            np.concatenate([np.asarray(in_maps[c][nm]) for c in range(n_cores)], axis=0),
            shd,
        )
        for nm in in_names
    ]
    big_zeros = [np.concatenate([z] * n_cores, axis=0) for z in zero_outs]

    best = None
    out_arrs = None
    for _ in range(max(1, iters)):
        zo = [jax.device_put(z, shd) for z in big_zeros]
        jax.block_until_ready(zo)
        jax.block_until_ready(concat_in)
        t0 = _time.perf_counter()
        out_arrs = sharded(*concat_in, *zo)
        jax.block_until_ready(out_arrs)
        dt = _time.perf_counter() - t0
        best = dt if best is None else min(best, dt)

    _LAST_RUN.clear()
    _LAST_RUN.update(
        sharded=sharded, concat_in=concat_in, big_zeros=big_zeros, shd=shd
    )

    results = []
    for c in range(n_cores):
        d = {}
        for i, nm in enumerate(out_names):
            full = np.asarray(out_arrs[i])
            per = full.shape[0] // n_cores
            d[nm] = full[c * per : (c + 1) * per]
        results.append(d)
    return results, best


_LAST_RUN = {}


def measure_exec_ns(m_lo=4, m_hi=8, reps=3):
    """Slope-based per-exec time: wall(M back-to-back launches) is
    overhead + M*exec, so the marginal cost between M=m_lo and M=m_hi
    cancels the (noisy) per-launch dispatch constant."""
    import time as _time
    import jax

    if not _LAST_RUN:
        return None
    sharded = _LAST_RUN["sharded"]
    concat_in = _LAST_RUN["concat_in"]
    big_zeros = _LAST_RUN["big_zeros"]
    shd = _LAST_RUN["shd"]
    best = {}
    for _ in range(reps):
        for M in (m_lo, m_hi):
            zos = [[jax.device_put(z, shd) for z in big_zeros] for _ in range(M)]
            for zo in zos:
                jax.block_until_ready(zo)
            t0 = _time.perf_counter()
            outs = [sharded(*concat_in, *zo) for zo in zos]
            jax.block_until_ready(outs)
            dt = _time.perf_counter() - t0
            if M not in best or dt < best[M]:
                best[M] = dt
    slope = (best[m_hi] - best[m_lo]) / (m_hi - m_lo)
    return int(slope * 1e9)


def _build_tiny():
    """Trivial kernel used to calibrate per-dispatch overhead."""
    nc = bacc.Bacc(None, target_bir_lowering=False)
    a = nc.dram_tensor("a", [P, P], F32, kind="ExternalInput")
    o = nc.dram_tensor("o", [P, P], F32, kind="ExternalOutput")
    with tile.TileContext(nc) as tc, ExitStack() as ctx:
        pool = ctx.enter_context(tc.tile_pool(name="p", bufs=1))
        t = pool.tile([P, P], F32)
        nc.sync.dma_start(t[:], a[:])
        nc.sync.dma_start(o[:], t[:])
    nc.finalize()
    return nc


def dispatch_baseline_ns(iters=5):
    nc = _CACHE.get("tiny")
    if nc is None:
        nc = _CACHE["tiny"] = _build_tiny()
    a = np.zeros((P, P), np.float32)
    _, best = _run_spmd_timed(nc, [{"a": a}] * NCORES, iters=iters)
    return int(best * 1e9)


def make_inputs(inp):
    emb = inp["emb"].astype(np.float32)
    common = {
        "tab_f": _pack_table(emb, inp["enc_Wih_l0f"], inp["enc_bih_l0f"] + inp["enc_bhh_l0f"]),
        "tab_b": _pack_table(emb, inp["enc_Wih_l0b"], inp["enc_bih_l0b"] + inp["enc_bhh_l0b"]),
        "tab_d": _pack_table(
            emb, inp["dec_Wih_l0"][:, :E], inp["dec_bih_l0"] + inp["dec_bhh_l0"]
        ),
        "whhT_l0f": _pack_whhT(inp["enc_Whh_l0f"]),
        "whhT_l0b": _pack_whhT(inp["enc_Whh_l0b"]),
        "whhT_l1f": _pack_whhT(inp["enc_Whh_l1f"]),
        "whhT_l1b": _pack_whhT(inp["enc_Whh_l1b"]),
        "whhT_d0": _pack_whhT(inp["dec_Whh_l0"]),
        "whhT_d1": _pack_whhT(inp["dec_Whh_l1"]),
        "wihT_l1f": _pack_wihT(inp["enc_Wih_l1f"], 0, 8),
        "wihT_l1b": _pack_wihT(inp["enc_Wih_l1b"], 0, 8),
        "wihT_d0e": _pack_wihT(inp["dec_Wih_l0"], E, 8),
        "wihT_d1": _pack_wihT(inp["dec_Wih_l1"], 0, NK),
        "owT": np.concatenate(
            [inp["out_W"][:, k * P : (k + 1) * P].T for k in range(NK)], axis=1
        ).astype(np.float16),
        "bias_l1f": _pack_bias(inp["enc_bih_l1f"], inp["enc_bhh_l1f"]),
        "bias_l1b": _pack_bias(inp["enc_bih_l1b"], inp["enc_bhh_l1b"]),
        "bias_d1": _pack_bias(inp["dec_bih_l1"], inp["dec_bhh_l1"]),
        "bias_out": inp["out_b"].astype(np.float32).reshape(P, 1),
        "ident": np.eye(P, dtype=np.float16),
    }

    x = np.asarray(inp["x"])
    T = x.shape[1]
    TOK = BL * T
    in_maps = []
    for c in range(NCORES):
        xl = x[c * BL : (c + 1) * BL].astype(np.int64)  # [BL, T]
        oh = np.zeros((V, TOK), np.float16)
        oh[xl.T.reshape(-1), np.arange(TOK)] = 1.0  # col j = t*BL+b
        in_maps.append({**common, "onehotT": oh})
    return in_maps


def kernel(**inp):
    x = np.asarray(inp["x"])
    B_, T = x.shape
    assert B_ == B
    TOK = BL * T

    key = T
    if key not in _CACHE:
        nc = bacc.Bacc(None, target_bir_lowering=False)
        build_model(nc, T)
        _CACHE[key] = nc
    nc = _CACHE[key]

    in_maps = make_inputs(inp)
    results, best_s = _run_spmd_timed(nc, in_maps, iters=3)
    global LAST_EXEC_NS
    LAST_EXEC_NS = int(best_s * 1e9)
    global LAST_RAW_NS
    LAST_RAW_NS = int(best_s * 1e9)

    out = np.empty((B, T, V), np.float32)
    for c in range(NCORES):
        lt = results[c]["logitsT"]  # [V, TOK]
        out[c * BL : (c + 1) * BL] = lt.reshape(V, T, BL).transpose(2, 1, 0)
    return out
